# revision 1
# baseline (speedup 1.0000x reference)
"""Trainium2 Bass kernel for nn_AttentionBlock (GroupNorm + per-position
head-axis attention + proj + residual).

Sharding: data-parallel over batch B=16 -> 2 batches per core x 8 cores.
Each core runs an identical program on its x-shard [2, 512, 4096] plus
replicated (host-preprocessed) weights, and writes its out-shard.

Per-core pipeline:
  1. GroupNorm(32): bn_stats per partition over N; cross-partition group
     aggregation via two tiny SBUF->SBUF DMA gathers (DMA crosses
     partitions); normalize on ACT with per-partition scale/bias.
  2. QKV: out[n, o] via PE with h-block stationary -> QKV arrives N-major.
     h is normalized into two half-batch column groups (low half
     double-buffered) so consecutive batches overlap.
  3. Attention (N-major, per 128-position block): logits/AV as broadcast
     elementwise multiplies (bf16 unit-stride so the DVE 2x mode applies;
     the V weight columns are host-permuted to [d*8+g] for this), with the
     d- and g-reductions done as in-place halving add-trees (adds get 2x
     mode; InstTensorReduce would run 1x). Softmax skips max-subtraction
     (logits are O(1) by construction); Exp on ACT with the 1/8 scale
     folded in. The logits multiply runs on GPSIMD for 2/3 of blocks.
  4. O transposed back to C-major via PE transpose; proj matmul on PE
     consumes a 3-deep ring of per-chunk OT tiles; residual-add fused into
     the PSUM->SBUF eviction on DVE (x re-DMA'd per chunk); DMA out.

Host-side preprocessing: weight transposes + bf16 casts + V-column permute.
If qkv_b is nonzero the kernel emits bias adds (specialized at trace; the
benchmark uses zero biases).

_cap_sync_waits: this walrus build accepts only ONE sync wait per compute
instruction; Tile emits more. The pass hoists excess waits onto same-engine
InstNoOps inserted immediately before the offender.
"""

import os

import numpy as np
import ml_dtypes

import concourse.bass as bass
import concourse.mybir as mybir
import concourse.tile as tile
from concourse.bass_utils import run_bass_kernel_spmd

F32 = mybir.dt.float32
BF16 = mybir.dt.bfloat16

B, C, HH, WW = 16, 512, 64, 64
N = HH * WW            # 4096
NB = 2                 # batches per core
NCORES = 8
NH, HD = 8, 64         # heads, head dim
GROUPS = 32
GSIZE = C // GROUPS    # 16 channels per group
EPS = 1e-5
CT = C // 128          # 4 channel tiles
OT3 = 3 * C // 512     # 3 o-chunks of 512 in qkv
NBLK = N // 128        # 32 position blocks per batch

AX = mybir.AxisListType
ALU = mybir.AluOpType
ACTF = mybir.ActivationFunctionType


def _bc(t, dims):
    """AP over tile/AP `t` with explicit free [step,count] dims (elem units)."""
    return bass.AP(tensor=t.tensor, offset=t.offset,
                   ap=[list(t.ap[0])] + [list(d) for d in dims])


def _cap_sync_waits(nc):
    """Walrus (this neuronxcc) allows at most 2 sync waits per compute
    instruction and is stricter still for some DMA structs. Tile can emit
    more. Hoist the excess onto a same-engine InstNoOp inserted immediately
    before the offender — the waits still complete before it executes."""
    import bass_rust
    n = 0
    for f in nc.m.functions:
        for blk in f.blocks:
            il = blk.instructions
            i = 0
            while i < len(il):
                ins = il[i]
                si = getattr(ins, "sync_info", None)
                if si is not None and si.on_wait and len(si.on_wait) > 1:
                    waits = list(si.on_wait)
                    for w in waits[:-1]:
                        nop = mybir.InstNoOp(name=f"W-abs-{n}", ins=[], outs=[])
                        n += 1
                        nop.engine = ins.engine
                        nop.sync_info = bass_rust.SyncInfo(on_wait=[w],
                                                           on_update=[])
                        il.insert(i, nop)
                        i += 1
                    si.on_wait = waits[-1:]
                i += 1
    return n


def build_kernel(nb=NB, nblk=NBLK, qk_bias=False, debug=False):
    n = nblk * 128
    cs = min(512, n)       # proj/residual n-chunk
    nch = n // cs
    nc = bass.Bass()
    dbg = {}
    if debug:
        dbg["h"] = nc.dram_tensor("dbg_h", [C, n], F32, kind="ExternalOutput")
        dbg["qkv"] = nc.dram_tensor("dbg_qkv", [128, 3 * C], F32, kind="ExternalOutput")
        dbg["s"] = nc.dram_tensor("dbg_s", [128, NH * NH], F32, kind="ExternalOutput")
        dbg["a"] = nc.dram_tensor("dbg_a", [128, NH * NH], F32, kind="ExternalOutput")
        dbg["o"] = nc.dram_tensor("dbg_o", [128, C], F32, kind="ExternalOutput")
        dbg["otsb"] = nc.dram_tensor("dbg_otsb", [C, n], F32, kind="ExternalOutput")

    x_d = nc.dram_tensor("x", [nb, C, n], F32, kind="ExternalInput")
    wqkvT_d = nc.dram_tensor("wqkvT", [C, 3 * C], BF16, kind="ExternalInput")
    pwT_d = nc.dram_tensor("pwT", [C, C], BF16, kind="ExternalInput")
    normw_d = nc.dram_tensor("normw", [C], F32, kind="ExternalInput")
    normb_d = nc.dram_tensor("normb", [C], F32, kind="ExternalInput")
    qkvb_d = nc.dram_tensor("qkvb", [3 * C], F32, kind="ExternalInput")
    pbeff_d = nc.dram_tensor("pbeff", [C], F32, kind="ExternalInput")
    ident_d = nc.dram_tensor("ident", [128, 128], BF16, kind="ExternalInput")
    out_d = nc.dram_tensor("out", [nb, C, n], F32, kind="ExternalOutput")

    with tile.TileContext(nc) as tc:
        with (
            tc.tile_pool(name="consts", bufs=1) as consts,
            tc.tile_pool(name="xpool", bufs=1) as xpool,
            tc.tile_pool(name="hlo", bufs=2) as hlo,
            tc.tile_pool(name="hhi", bufs=1) as hhi,
            tc.tile_pool(name="otr", bufs=3) as otr,
            tc.tile_pool(name="stats", bufs=2) as stats,
            tc.tile_pool(name="scb", bufs=4) as scb,
            tc.tile_pool(name="qkvsb", bufs=3) as qkvsb,
            tc.tile_pool(name="upool", bufs=4) as upool,
            tc.tile_pool(name="spool", bufs=4) as spool,
            tc.tile_pool(name="opool", bufs=4) as opool,
            tc.tile_pool(name="outsb", bufs=2) as outsb,
            tc.tile_pool(name="pqkv", bufs=4, space="PSUM") as pqkv,   # 6 banks
            tc.tile_pool(name="pmm", bufs=2, space="PSUM") as pmm,     # 2 banks
        ):
            # ---- constants / weights in SBUF ----
            wqkvT = []
            for c in range(CT):
                t = consts.tile([128, 3 * C], BF16, tag=f"wq{c}")
                nc.sync.dma_start(out=t, in_=wqkvT_d[c * 128:(c + 1) * 128, :])
                wqkvT.append(t)
            pwT = []
            for o in range(CT):
                t = consts.tile([128, C], BF16, tag=f"pw{o}")
                nc.sync.dma_start(out=t, in_=pwT_d[o * 128:(o + 1) * 128, :])
                pwT.append(t)
            ident = consts.tile([128, 128], BF16, tag="ident")
            nc.sync.dma_start(out=ident, in_=ident_d[:, :])
            nwt, nbt, pbt = [], [], []
            for c in range(CT):
                sl = slice(c * 128, (c + 1) * 128)
                t1 = consts.tile([128, 1], F32, tag=f"nw{c}")
                nc.sync.dma_start(out=t1, in_=normw_d[sl].rearrange("(p u) -> p u", u=1))
                nwt.append(t1)
                t2 = consts.tile([128, 1], F32, tag=f"nb{c}")
                nc.sync.dma_start(out=t2, in_=normb_d[sl].rearrange("(p u) -> p u", u=1))
                nbt.append(t2)
                t3 = consts.tile([128, 1], F32, tag=f"pb{c}")
                nc.sync.dma_start(out=t3, in_=pbeff_d[sl].rearrange("(p u) -> p u", u=1))
                pbt.append(t3)
            epst = consts.tile([1, 1], F32, tag="eps")
            nc.vector.memset(epst, 256.0 * EPS)
            qkbias = None
            if qk_bias:
                qkbias = consts.tile([128, 3 * C], F32, tag="qkb")
                nc.sync.dma_start(
                    out=qkbias,
                    in_=bass.AP(tensor=qkvb_d.ap().tensor, offset=0,
                                ap=[[0, 128], [1, 3 * C]]))

            hb = max(1, nblk // 2)          # blocks per half
            nh2 = hb * 128

            def emit_head(b):
                    # ---------- load x, GroupNorm ----------
                    xt, scale_t, bias_t = [], [], []  # noqa
                    for c in range(CT):
                        t = xpool.tile([128, n], F32, tag=f"x{c}")
                        nc.sync.dma_start(out=t, in_=x_d[b, c * 128:(c + 1) * 128, :])
                        xt.append(t)
                    for c in range(CT):
                        nsub = max(1, n // 512)
                        sd = nc.vector.BN_STATS_DIM
                        st = stats.tile([128, nsub, sd], F32, tag="bnst")
                        xv = xt[c].rearrange("p (s f) -> p s f", s=nsub)
                        for s in range(nsub):
                            nc.vector.bn_stats(out=st[:, s, :], in_=xv[:, s, :])
                        mv = stats.tile([128, nc.vector.BN_AGGR_DIM], F32, tag="bnmv")
                        nc.vector.bn_aggr(out=mv, in_=st)
                        # st2: col0 = mean, col1 = E[x^2] = var + mean^2
                        st2 = stats.tile([128, 2], F32, tag="st2")
                        nc.vector.tensor_copy(out=st2[:, 0:1], in_=mv[:, 0:1])
                        nc.vector.scalar_tensor_tensor(
                            out=st2[:, 1:2], in0=mv[:, 0:1], scalar=mv[:, 0:1],
                            in1=mv[:, 1:2], op0=ALU.mult, op1=ALU.add)
                        # gather all 128 partitions' stats onto one partition (DMA
                        # crosses partitions; avoids PE for the group aggregation)
                        stT = stats.tile([1, 256], F32, tag="stT")
                        nc.gpsimd.dma_start(out=stT, in_=st2)
                        # per-group sums over the 16 channels: [1, 8, 2]
                        gsum = stats.tile([1, 16], F32, tag="gsum")
                        nc.vector.tensor_reduce(
                            out=gsum.rearrange("p (g c) -> p g c", g=8),
                            in_=_bc(stT, [(32, 8), (1, 2), (2, 16)]),
                            axis=AX.X, op=ALU.add)
                        gm = _bc(gsum, [(2, 8)])           # sum of means      [1,8]
                        ge = bass.AP(tensor=gsum.tensor, offset=gsum.offset + 1,
                                     ap=[list(gsum.ap[0])] + [[2, 8]])  # sum E[x^2]
                        m2 = stats.tile([1, 8], F32, tag="m2")
                        nc.vector.tensor_mul(m2, gm, gm)
                        # 256*var = 16*sum_ex2 - (sum_mean)^2
                        v256 = stats.tile([1, 8], F32, tag="v256")
                        nc.vector.scalar_tensor_tensor(
                            out=v256, in0=ge, scalar=16.0, in1=m2,
                            op0=ALU.mult, op1=ALU.subtract)
                        sg = stats.tile([1, 8], F32, tag="sg")
                        nc.scalar.activation(out=sg, in_=v256, func=ACTF.Sqrt,
                                             scale=1.0, bias=epst)   # sqrt(256(var+eps))
                        rg = stats.tile([1, 8], F32, tag="rg")
                        nc.vector.reciprocal(out=rg, in_=sg)          # rstd/16
                        # broadcast to 128 channel slots, interleaved (mean, rstd)
                        sb2 = stats.tile([1, 256], F32, tag="sb2")
                        nc.vector.tensor_scalar(
                            out=_bc(sb2, [(32, 8), (2, 16)]),
                            in0=_bc(gsum, [(2, 8), (0, 16)]), scalar1=1.0 / 16.0,
                            scalar2=None, op0=ALU.mult)
                        nc.vector.tensor_scalar(
                            out=bass.AP(tensor=sb2.tensor, offset=sb2.offset + 1,
                                        ap=[list(sb2.ap[0])] + [[32, 8], [2, 16]]),
                            in0=_bc(rg, [(1, 8), (0, 16)]), scalar1=16.0,
                            scalar2=None, op0=ALU.mult)
                        pb2 = stats.tile([128, 2], F32, tag="pb2")
                        nc.gpsimd.dma_start(out=pb2, in_=sb2)
                        sc = scb.tile([128, 1], F32, tag="sc", name="sc")
                        bi = scb.tile([128, 1], F32, tag="bi", name="bi")
                        tmp = stats.tile([128, 1], F32, tag="tmp")
                        nc.vector.tensor_mul(sc, pb2[:, 1:2], nwt[c])
                        nc.vector.tensor_mul(tmp, pb2[:, 0:1], sc)
                        nc.vector.tensor_sub(bi, nbt[c], tmp)
                        scale_t.append(sc)
                        bias_t.append(bi)
                    return xt, scale_t, bias_t

            def emit_norm(b, half, xt, scale_t, bias_t):
                hs = slice(half * nh2, min(n, (half + 1) * nh2))
                out = []
                for c in range(CT):
                    hp = hlo if half == 0 else hhi
                    t = hp.tile([128, nh2], BF16, tag=f"h{half}_{c}",
                                name=f"h{half}_{c}")
                    nc.scalar.activation(out=t, in_=xt[c][:, hs],
                                         func=ACTF.Identity,
                                         bias=bias_t[c], scale=scale_t[c])
                    out.append(t)
                    if debug and b == 0:
                        hf = stats.tile([128, nh2], F32, tag="dbgh", name="hf")
                        nc.vector.tensor_copy(out=hf, in_=t)
                        nc.sync.dma_start(
                            out=dbg["h"][c * 128:(c + 1) * 128, hs], in_=hf)
                return out

            for b in range(nb):
                xt, scale_t, bias_t = emit_head(b)
                ht = [emit_norm(b, 0, xt, scale_t, bias_t), None]
                if nblk > 1:
                    ht[1] = emit_norm(b, 1, xt, scale_t, bias_t)
                else:
                    ht[1] = ht[0]
                # ---------- per 128-position block ----------
                bpc = cs // 128
                otc = None
                for blk in range(nblk):
                    ns = slice(blk * 128, (blk + 1) * 128)
                    if blk % bpc == 0:
                        otc = [otr.tile([128, cs], BF16, tag=f"otr{ob}",
                                        name=f"otr{ob}") for ob in range(CT)]
                    half = min(blk // hb, 1)
                    hslice = slice(blk * 128 - half * nh2, (blk + 1) * 128 - half * nh2)
                    pqc = [pqkv.tile([128, 512], F32, tag="pq", name=f"pq{oc}")
                           for oc in range(OT3)]
                    for c in range(CT):
                        lhsT = ht[half][c][:, hslice]
                        for oc in range(OT3):
                            nc.tensor.matmul(
                                pqc[oc], lhsT,
                                wqkvT[c][:, oc * 512:(oc + 1) * 512],
                                start=(c == 0), stop=(c == CT - 1))
                    qkv = qkvsb.tile([128, 3 * C], BF16, tag="qkv")
                    for oc in range(OT3):
                        if qkbias is not None:
                            nc.vector.tensor_add(
                                out=qkv[:, oc * 512:(oc + 1) * 512], in0=pqc[oc],
                                in1=qkbias[:, oc * 512:(oc + 1) * 512])
                        else:
                            nc.scalar.copy(
                                out=qkv[:, oc * 512:(oc + 1) * 512], in_=pqc[oc])

                    q = qkv[:, 0:512]
                    k = qkv[:, 512:1024]
                    v = qkv[:, 1024:1536]

                    # logits: U1[(h,g,d)] = q[h,d] * k[g,d]
                    u1 = upool.tile([128, NH * NH * HD], BF16, tag="u")
                    u1eng = nc.gpsimd if True else nc.vector
                    u1eng.tensor_tensor(
                        out=u1.rearrange("p (h g d) -> p h g d", h=NH, g=NH),
                        in0=_bc(q, [(HD, NH), (0, NH), (1, HD)]),
                        in1=_bc(k, [(0, NH), (HD, NH), (1, HD)]),
                        op=ALU.mult)
                    u1v = u1.rearrange("p (a d) -> p a d", d=HD)
                    w = HD
                    while w > 2:
                        nc.vector.tensor_tensor(
                            out=u1v[:, :, 0:w // 2], in0=u1v[:, :, 0:w // 2],
                            in1=u1v[:, :, w // 2:w], op=ALU.add)
                        w //= 2
                    s_l = spool.tile([128, NH * NH], F32, tag="s")
                    nc.vector.tensor_tensor(
                        out=s_l.rearrange("p (a u) -> p a u", u=1),
                        in0=u1v[:, :, 0:1], in1=u1v[:, :, 1:2], op=ALU.add)
                    # softmax over g: E = exp(S/8); logits bounded so no max-sub
                    e_l = spool.tile([128, NH * NH], BF16, tag="e")
                    nc.scalar.activation(out=e_l, in_=s_l, func=ACTF.Exp,
                                         scale=0.125)
                    d_l = spool.tile([128, NH], F32, tag="d")
                    nc.vector.tensor_reduce(
                        out=d_l, in_=e_l.rearrange("p (h g) -> p h g", g=NH),
                        axis=AX.X, op=ALU.add)
                    r_l = spool.tile([128, NH], F32, tag="r")
                    nc.vector.reciprocal(out=r_l, in_=d_l)
                    a_l = spool.tile([128, NH * NH], BF16, tag="a")
                    nc.vector.tensor_tensor(
                        out=a_l.rearrange("p (h g) -> p h g", g=NH),
                        in0=e_l.rearrange("p (h g) -> p h g", g=NH),
                        in1=_bc(r_l, [(1, NH), (0, NH)]),
                        op=ALU.mult)
                    # AV: U2[(h,d,g)] = A[h,g] * V'[d,g]; O = sum_g
                    # (V columns host-permuted to [d*8+g] so both reads are
                    # unit-stride innermost -> DVE 2x mode)
                    u2 = upool.tile([128, NH * HD * NH], BF16, tag="u")
                    nc.vector.tensor_tensor(
                        out=u2.rearrange("p (h d g) -> p h d g", h=NH, d=HD),
                        in0=_bc(a_l, [(NH, NH), (0, HD), (1, NH)]),
                        in1=_bc(v, [(0, NH), (NH, HD), (1, NH)]),
                        op=ALU.mult)
                    # O = sum_g via in-place halving adds (2x-mode eligible)
                    uv = u2.rearrange("p (a g) -> p a g", g=NH)
                    w = NH
                    while w > 2:
                        nc.vector.tensor_tensor(
                            out=uv[:, :, 0:w // 2], in0=uv[:, :, 0:w // 2],
                            in1=uv[:, :, w // 2:w], op=ALU.add)
                        w //= 2
                    o_l = opool.tile([128, C], BF16, tag="o")
                    nc.vector.tensor_tensor(
                        out=o_l.rearrange("p (a u) -> p a u", u=1),
                        in0=uv[:, :, 0:1], in1=uv[:, :, 1:2], op=ALU.add)
                    if debug and b == 0 and blk == 0:
                        for nm, src in (("qkv", qkv), ("s", s_l), ("a", a_l), ("o", o_l)):
                            ff = stats.tile(list(src.shape), F32, tag=f"dbg{nm}",
                                            name=f"f{nm}")
                            nc.vector.tensor_copy(out=ff, in_=src)
                            nc.sync.dma_start(out=dbg[nm][:, :], in_=ff)
                    # transpose O back to C-major
                    pt = pmm.tile([128, 512], BF16, tag="pt")
                    for ob in range(CT):
                        nc.tensor.transpose(pt[:, ob * 128:(ob + 1) * 128],
                                            o_l[:, ob * 128:(ob + 1) * 128], ident)
                    for ob in range(CT):
                        nc.scalar.copy(
                            out=otc[ob][:, (blk % bpc) * 128:(blk % bpc + 1) * 128],
                            in_=pt[:, ob * 128:(ob + 1) * 128])

                    # proj + residual for chunk j as soon as its 4 blocks of
                    # OT columns exist (x re-DMA'd per chunk; x tiles free
                    # after norm)
                    if (blk + 1) % bpc == 0:
                        j = blk // bpc
                        ncs = slice(j * cs, (j + 1) * cs)
                        for c in range(CT):
                            xr = outsb.tile([128, cs], F32, tag="xr", name="xr")
                            nc.sync.dma_start(out=xr,
                                              in_=x_d[b, c * 128:(c + 1) * 128, ncs])
                            py = pmm.tile([128, cs], F32, tag="py", name="py")
                            for ob in range(CT):
                                nc.tensor.matmul(py,
                                                 pwT[ob][:, c * 128:(c + 1) * 128],
                                                 otc[ob],
                                                 start=(ob == 0), stop=(ob == CT - 1))
                            ot = outsb.tile([128, cs], F32, tag="out", name="ot")
                            nc.vector.scalar_tensor_tensor(
                                out=ot, in0=py, scalar=pbt[c], in1=xr,
                                op0=ALU.add, op1=ALU.add)
                            nc.sync.dma_start(
                                out=out_d[b, c * 128:(c + 1) * 128, ncs], in_=ot)
    return nc


_CACHE = {}


def host_inputs(norm_w, norm_b, qkv_w, qkv_b, proj_w, proj_b):
    """Host-side weight preprocessing -> the kernel's shared input tensors."""
    bf = ml_dtypes.bfloat16
    # V-part column permutation: store V as [d*8+g] so the AV multiply reads
    # both operands at unit stride (DVE 2x mode).
    vperm = np.arange(3 * C)
    g_i, d_i = np.meshgrid(np.arange(NH), np.arange(HD), indexing="ij")
    vperm[2 * C:] = 2 * C + (d_i * NH + g_i).reshape(-1)   # old[g*64+d] -> new pos
    inv = np.empty_like(vperm)
    inv[vperm] = np.arange(3 * C)
    wq_p = qkv_w[inv]        # new column j holds old channel inv[j]
    qkvb_p = np.ascontiguousarray(qkv_b[inv])
    wqkvT = np.ascontiguousarray(wq_p.T).astype(bf)           # [C, 3C]
    pwT = np.ascontiguousarray(proj_w.T).astype(bf)           # [C(o), C(c)]
    ident = np.eye(128, dtype=np.float32).astype(bf)
    return dict(wqkvT=wqkvT, pwT=pwT,
                normw=np.asarray(norm_w, np.float32),
                normb=np.asarray(norm_b, np.float32),
                qkvb=qkvb_p, pbeff=np.asarray(proj_b, np.float32),
                ident=ident)


def kernel(x, norm_w, norm_b, qkv_w, qkv_b, proj_w, proj_b):
    x = np.asarray(x, np.float32)
    norm_w = np.asarray(norm_w, np.float32)
    norm_b = np.asarray(norm_b, np.float32)
    qkv_w = np.asarray(qkv_w, np.float32)
    qkv_b = np.asarray(qkv_b, np.float32)
    proj_w = np.asarray(proj_w, np.float32)
    proj_b = np.asarray(proj_b, np.float32)

    qk_bias = bool(np.any(qkv_b != 0))
    key = ("full", qk_bias)
    if key not in _CACHE:
        nc_new = build_kernel(qk_bias=qk_bias)
        _cap_sync_waits(nc_new)   # HW path only; CoreSim rejects bare NoOps
        _CACHE[key] = nc_new
    nc = _CACHE[key]

    shared = host_inputs(norm_w, norm_b, qkv_w, qkv_b, proj_w, proj_b)
    xs = x.reshape(B, C, N)
    in_maps = [dict(x=np.ascontiguousarray(xs[c * NB:(c + 1) * NB]), **shared)
               for c in range(NCORES)]
    res = run_bass_kernel_spmd(nc, in_maps, core_ids=list(range(NCORES)),
                               trace=bool(os.environ.get("KERNEL_TRACE")))
    global LAST_RES
    LAST_RES = res
    out = np.concatenate([res.results[c]["out"] for c in range(NCORES)], axis=0)
    return out.reshape(B, C, HH, WW).astype(np.float32)


LAST_RES = None



# revision 27
# speedup vs baseline: 1.1420x; 1.1420x over previous
"""Trainium2 Bass kernel for nn_AttentionBlock (GroupNorm + per-position
head-axis attention + proj + residual).

Sharding: data-parallel over batch B=16 -> 2 batches per core x 8 cores.
Each core runs an identical program on its x-shard [2, 512, 4096] plus
replicated (host-preprocessed) weights, and writes its out-shard.

Per-core pipeline:
  1. GroupNorm(32): bn_stats per partition over N; cross-partition group
     aggregation via two tiny SBUF->SBUF DMA gathers (DMA crosses
     partitions); normalize on ACT with per-partition scale/bias.
  2. QKV: out[n, o] via PE with h-block stationary -> QKV arrives N-major.
     h is normalized into two half-batch column groups (low half
     double-buffered) so consecutive batches overlap.
  3. Attention (N-major, per 128-position block): logits/AV as broadcast
     elementwise multiplies (bf16 unit-stride so the DVE 2x mode applies;
     the V weight columns are host-permuted to [d*8+g] for this), with the
     d- and g-reductions done as in-place halving add-trees (adds get 2x
     mode; InstTensorReduce would run 1x). Softmax skips max-subtraction
     (logits are O(1) by construction); Exp on ACT with the 1/8 scale
     folded in. The logits multiply runs on GPSIMD for 2/3 of blocks.
  4. O transposed back to C-major via PE transpose; proj matmul on PE
     consumes a 3-deep ring of per-chunk OT tiles; residual-add fused into
     the PSUM->SBUF eviction on DVE (x re-DMA'd per chunk); DMA out.

Host-side preprocessing: weight transposes + bf16 casts + V-column permute.
If qkv_b is nonzero the kernel emits bias adds (specialized at trace; the
benchmark uses zero biases).

_cap_sync_waits: this walrus build accepts only ONE sync wait per compute
instruction; Tile emits more. The pass hoists excess waits onto same-engine
InstNoOps inserted immediately before the offender.
"""

import os

import numpy as np
import ml_dtypes

import concourse.bass as bass
import concourse.mybir as mybir
import concourse.tile as tile
from concourse.bass_utils import run_bass_kernel_spmd

F32 = mybir.dt.float32
BF16 = mybir.dt.bfloat16

B, C, HH, WW = 16, 512, 64, 64
N = HH * WW            # 4096
NB = 2                 # batches per core
NCORES = 8
NH, HD = 8, 64         # heads, head dim
GROUPS = 32
GSIZE = C // GROUPS    # 16 channels per group
EPS = 1e-5
CT = C // 128          # 4 channel tiles
OT3 = 3 * C // 512     # 3 o-chunks of 512 in qkv
NBLK = N // 128        # 32 position blocks per batch

AX = mybir.AxisListType
ALU = mybir.AluOpType
ACTF = mybir.ActivationFunctionType

# blocks per batch (of NBLK=32) whose logits multiply runs on GPSIMD
NMULT = 29


def _bc(t, dims):
    """AP over tile/AP `t` with explicit free [step,count] dims (elem units)."""
    return bass.AP(tensor=t.tensor, offset=t.offset,
                   ap=[list(t.ap[0])] + [list(d) for d in dims])


def _cap_sync_waits(nc):
    """Walrus (this neuronxcc) allows at most 2 sync waits per compute
    instruction and is stricter still for some DMA structs. Tile can emit
    more. Hoist the excess onto a same-engine InstNoOp inserted immediately
    before the offender — the waits still complete before it executes."""
    import bass_rust
    n = 0
    for f in nc.m.functions:
        for blk in f.blocks:
            il = blk.instructions
            i = 0
            while i < len(il):
                ins = il[i]
                si = getattr(ins, "sync_info", None)
                if si is not None and si.on_wait and len(si.on_wait) > 1:
                    waits = list(si.on_wait)
                    for w in waits[:-1]:
                        nop = mybir.InstNoOp(name=f"W-abs-{n}", ins=[], outs=[])
                        n += 1
                        nop.engine = ins.engine
                        nop.sync_info = bass_rust.SyncInfo(on_wait=[w],
                                                           on_update=[])
                        il.insert(i, nop)
                        i += 1
                    si.on_wait = waits[-1:]
                i += 1
    return n


def build_kernel(nb=NB, nblk=NBLK, qk_bias=False, debug=False):
    n = nblk * 128
    cs = min(512, n)       # proj/residual n-chunk
    nch = n // cs
    nc = bass.Bass()
    dbg = {}
    if debug:
        dbg["h"] = nc.dram_tensor("dbg_h", [C, n], F32, kind="ExternalOutput")
        dbg["qkv"] = nc.dram_tensor("dbg_qkv", [128, 3 * C], F32, kind="ExternalOutput")
        dbg["s"] = nc.dram_tensor("dbg_s", [128, NH * NH], F32, kind="ExternalOutput")
        dbg["a"] = nc.dram_tensor("dbg_a", [128, NH * NH], F32, kind="ExternalOutput")
        dbg["o"] = nc.dram_tensor("dbg_o", [128, C], F32, kind="ExternalOutput")
        dbg["otsb"] = nc.dram_tensor("dbg_otsb", [C, n], F32, kind="ExternalOutput")

    x_d = nc.dram_tensor("x", [nb, C, n], F32, kind="ExternalInput")
    wqkvT_d = nc.dram_tensor("wqkvT", [C, 3 * C], BF16, kind="ExternalInput")
    pwT_d = nc.dram_tensor("pwT", [C, C], BF16, kind="ExternalInput")
    normw_d = nc.dram_tensor("normw", [C], F32, kind="ExternalInput")
    normb_d = nc.dram_tensor("normb", [C], F32, kind="ExternalInput")
    qkvb_d = nc.dram_tensor("qkvb", [3 * C], F32, kind="ExternalInput")
    pbeff_d = nc.dram_tensor("pbeff", [C], F32, kind="ExternalInput")
    ident_d = nc.dram_tensor("ident", [128, 128], BF16, kind="ExternalInput")
    out_d = nc.dram_tensor("out", [nb, C, n], F32, kind="ExternalOutput")

    with tile.TileContext(nc) as tc:
        with (
            tc.tile_pool(name="consts", bufs=1) as consts,
            tc.tile_pool(name="xpool", bufs=1) as xpool,
            tc.tile_pool(name="hlo", bufs=2) as hlo,
            tc.tile_pool(name="hhi", bufs=1) as hhi,
            tc.tile_pool(name="otr", bufs=3) as otr,
            tc.tile_pool(name="stats", bufs=2) as stats,
            tc.tile_pool(name="scb", bufs=4) as scb,
            tc.tile_pool(name="qkvsb", bufs=3) as qkvsb,
            tc.tile_pool(name="upool", bufs=4) as upool,
            tc.tile_pool(name="spool", bufs=4) as spool,
            tc.tile_pool(name="opool", bufs=4) as opool,
            tc.tile_pool(name="outsb", bufs=2) as outsb,
            tc.tile_pool(name="pqkv", bufs=2, space="PSUM") as pqkv,   # 6 banks
            tc.tile_pool(name="pmm", bufs=1, space="PSUM") as pmm,     # 1 bank
            tc.tile_pool(name="pmm2", bufs=1, space="PSUM") as pmm2,   # 1 bank
        ):
            def emit_xload(b):
                xt = []
                for c in range(CT):
                    t = xpool.tile([128, n], F32, tag=f"x{c}")
                    nc.sync.dma_start(out=t, in_=x_d[b, c * 128:(c + 1) * 128, :])
                    xt.append(t)
                return xt

            # batch 0's x DMAs go first so GroupNorm stats start immediately;
            # the weight loads below overlap with them.
            xt_cur = emit_xload(0)

            # ---- constants / weights in SBUF ----
            wqkvT = []
            for c in range(CT):
                t = consts.tile([128, 3 * C], BF16, tag=f"wq{c}")
                nc.sync.dma_start(out=t, in_=wqkvT_d[c * 128:(c + 1) * 128, :])
                wqkvT.append(t)
            pwT = []
            for o in range(CT):
                t = consts.tile([128, C], BF16, tag=f"pw{o}")
                nc.sync.dma_start(out=t, in_=pwT_d[o * 128:(o + 1) * 128, :])
                pwT.append(t)
            ident = consts.tile([128, 128], BF16, tag="ident")
            nc.sync.dma_start(out=ident, in_=ident_d[:, :])
            nwt, nbt, pbt = [], [], []
            for c in range(CT):
                sl = slice(c * 128, (c + 1) * 128)
                t1 = consts.tile([128, 1], F32, tag=f"nw{c}")
                nc.sync.dma_start(out=t1, in_=normw_d[sl].rearrange("(p u) -> p u", u=1))
                nwt.append(t1)
                t2 = consts.tile([128, 1], F32, tag=f"nb{c}")
                nc.sync.dma_start(out=t2, in_=normb_d[sl].rearrange("(p u) -> p u", u=1))
                nbt.append(t2)
                t3 = consts.tile([128, 1], F32, tag=f"pb{c}")
                nc.sync.dma_start(out=t3, in_=pbeff_d[sl].rearrange("(p u) -> p u", u=1))
                pbt.append(t3)
            epst = consts.tile([1, 1], F32, tag="eps")
            nc.vector.memset(epst, 256.0 * EPS)
            qkbias = None
            if qk_bias:
                qkbias = consts.tile([128, 3 * C], F32, tag="qkb")
                nc.sync.dma_start(
                    out=qkbias,
                    in_=bass.AP(tensor=qkvb_d.ap().tensor, offset=0,
                                ap=[[0, 128], [1, 3 * C]]))

            hb = max(1, nblk // 2)          # blocks per half
            nh2 = hb * 128

            def emit_stats(b, xt):
                    # ---------- GroupNorm stats ----------
                    scale_t, bias_t = [], []
                    for c in range(CT):
                        nsub = max(1, n // 512)
                        sd = nc.vector.BN_STATS_DIM
                        st = stats.tile([128, nsub, sd], F32, tag="bnst")
                        xv = xt[c].rearrange("p (s f) -> p s f", s=nsub)
                        for s in range(nsub):
                            nc.vector.bn_stats(out=st[:, s, :], in_=xv[:, s, :])
                        mv = stats.tile([128, nc.vector.BN_AGGR_DIM], F32, tag="bnmv")
                        nc.vector.bn_aggr(out=mv, in_=st)
                        # st2: col0 = mean, col1 = E[x^2] = var + mean^2
                        st2 = stats.tile([128, 2], F32, tag="st2")
                        nc.vector.tensor_copy(out=st2[:, 0:1], in_=mv[:, 0:1])
                        nc.vector.scalar_tensor_tensor(
                            out=st2[:, 1:2], in0=mv[:, 0:1], scalar=mv[:, 0:1],
                            in1=mv[:, 1:2], op0=ALU.mult, op1=ALU.add)
                        # gather all 128 partitions' stats onto one partition (DMA
                        # crosses partitions; avoids PE for the group aggregation)
                        stT = stats.tile([1, 256], F32, tag="stT")
                        nc.sync.dma_start(out=stT, in_=st2)
                        # per-group sums over the 16 channels: [1, 8, 2]
                        gsum = stats.tile([1, 16], F32, tag="gsum")
                        nc.vector.tensor_reduce(
                            out=gsum.rearrange("p (g c) -> p g c", g=8),
                            in_=_bc(stT, [(32, 8), (1, 2), (2, 16)]),
                            axis=AX.X, op=ALU.add)
                        gm = _bc(gsum, [(2, 8)])           # sum of means      [1,8]
                        ge = bass.AP(tensor=gsum.tensor, offset=gsum.offset + 1,
                                     ap=[list(gsum.ap[0])] + [[2, 8]])  # sum E[x^2]
                        m2 = stats.tile([1, 8], F32, tag="m2")
                        nc.vector.tensor_mul(m2, gm, gm)
                        # 256*var = 16*sum_ex2 - (sum_mean)^2
                        v256 = stats.tile([1, 8], F32, tag="v256")
                        nc.vector.scalar_tensor_tensor(
                            out=v256, in0=ge, scalar=16.0, in1=m2,
                            op0=ALU.mult, op1=ALU.subtract)
                        sg = stats.tile([1, 8], F32, tag="sg")
                        nc.scalar.activation(out=sg, in_=v256, func=ACTF.Sqrt,
                                             scale=1.0, bias=epst)   # sqrt(256(var+eps))
                        rg = stats.tile([1, 8], F32, tag="rg")
                        nc.vector.reciprocal(out=rg, in_=sg)          # rstd/16
                        # broadcast to 128 channel slots, interleaved (mean, rstd)
                        sb2 = stats.tile([1, 256], F32, tag="sb2")
                        nc.vector.tensor_scalar(
                            out=_bc(sb2, [(32, 8), (2, 16)]),
                            in0=_bc(gsum, [(2, 8), (0, 16)]), scalar1=1.0 / 16.0,
                            scalar2=None, op0=ALU.mult)
                        nc.vector.tensor_scalar(
                            out=bass.AP(tensor=sb2.tensor, offset=sb2.offset + 1,
                                        ap=[list(sb2.ap[0])] + [[32, 8], [2, 16]]),
                            in0=_bc(rg, [(1, 8), (0, 16)]), scalar1=16.0,
                            scalar2=None, op0=ALU.mult)
                        pb2 = stats.tile([128, 2], F32, tag="pb2")
                        nc.sync.dma_start(out=pb2, in_=sb2)
                        sc = scb.tile([128, 1], F32, tag="sc", name="sc")
                        bi = scb.tile([128, 1], F32, tag="bi", name="bi")
                        tmp = stats.tile([128, 1], F32, tag="tmp")
                        nc.vector.tensor_mul(sc, pb2[:, 1:2], nwt[c])
                        nc.vector.tensor_mul(tmp, pb2[:, 0:1], sc)
                        nc.vector.tensor_sub(bi, nbt[c], tmp)
                        scale_t.append(sc)
                        bias_t.append(bi)
                    return scale_t, bias_t

            def emit_norm(b, half, xt, scale_t, bias_t):
                hs = slice(half * nh2, min(n, (half + 1) * nh2))
                out = []
                for c in range(CT):
                    hp = hlo if half == 0 else hhi
                    t = hp.tile([128, nh2], BF16, tag=f"h{half}_{c}",
                                name=f"h{half}_{c}")
                    nc.scalar.activation(out=t, in_=xt[c][:, hs],
                                         func=ACTF.Identity,
                                         bias=bias_t[c], scale=scale_t[c])
                    out.append(t)
                    if debug and b == 0:
                        hf = stats.tile([128, nh2], F32, tag="dbgh", name="hf")
                        nc.vector.tensor_copy(out=hf, in_=t)
                        nc.sync.dma_start(
                            out=dbg["h"][c * 128:(c + 1) * 128, hs], in_=hf)
                return out

            st_cur = emit_stats(0, xt_cur)
            ht_cur = [emit_norm(0, 0, xt_cur, *st_cur), None]
            ht_cur[1] = emit_norm(0, 1, xt_cur, *st_cur) if nblk > 1 else ht_cur[0]
            for b in range(nb):
                xt, (scale_t, bias_t), ht = xt_cur, st_cur, ht_cur
                # ---------- per 128-position block ----------
                # The next batch's x load / stats / norm are emitted at fixed
                # points inside this loop so its head overlaps this batch's
                # attention tail (engine queues are in-order).
                nxt = {}
                bpc = cs // 128
                state = {"otcb": None}

                def emit_tail(tblk, qkv, s_l):
                    """Stage B of block tblk: softmax tail + AV + transpose +
                    proj. Emitted one block late so the ACT stream never
                    blocks the next block's qkv eviction behind exp()."""
                    v = qkv[:, 1024:1536]
                    if tblk % bpc == 0:
                        state["otcb"] = otr.tile([128, CT * cs], BF16,
                                                 tag="otr", name="otr")
                    otcb = state["otcb"]
                    # softmax over g: E = exp(S/8); logits bounded, no max-sub
                    e_l = spool.tile([128, NH * NH], BF16, tag="e")
                    nc.scalar.activation(out=e_l, in_=s_l, func=ACTF.Exp,
                                         scale=0.125)
                    d_l = spool.tile([128, NH], F32, tag="d")
                    nc.vector.tensor_reduce(
                        out=d_l, in_=e_l.rearrange("p (h g) -> p h g", g=NH),
                        axis=AX.X, op=ALU.add)
                    r_l = spool.tile([128, NH], F32, tag="r")
                    nc.vector.reciprocal(out=r_l, in_=d_l)
                    a_l = spool.tile([128, NH * NH], BF16, tag="a")
                    nc.vector.tensor_tensor(
                        out=a_l.rearrange("p (h g) -> p h g", g=NH),
                        in0=e_l.rearrange("p (h g) -> p h g", g=NH),
                        in1=_bc(r_l, [(1, NH), (0, NH)]),
                        op=ALU.mult)
                    # AV: U2[(h,d,g)] = A[h,g] * V'[d,g]; O = sum_g
                    # (V columns host-permuted to [d*8+g] so both reads are
                    # unit-stride innermost -> DVE 2x mode)
                    u2 = upool.tile([128, NH * HD * NH], BF16, tag="u")
                    nc.vector.tensor_tensor(
                        out=u2.rearrange("p (h d g) -> p h d g", h=NH, d=HD),
                        in0=_bc(a_l, [(NH, NH), (0, HD), (1, NH)]),
                        in1=_bc(v, [(0, NH), (NH, HD), (1, NH)]),
                        op=ALU.mult)
                    # O = sum_g via in-place halving adds (2x-mode eligible)
                    uv = u2.rearrange("p (a g) -> p a g", g=NH)
                    w = NH
                    while w > 2:
                        nc.vector.tensor_tensor(
                            out=uv[:, :, 0:w // 2], in0=uv[:, :, 0:w // 2],
                            in1=uv[:, :, w // 2:w], op=ALU.add)
                        w //= 2
                    o_l = opool.tile([128, C], BF16, tag="o")
                    nc.vector.tensor_tensor(
                        out=o_l.rearrange("p (a u) -> p a u", u=1),
                        in0=uv[:, :, 0:1], in1=uv[:, :, 1:2], op=ALU.add)
                    if debug and b == 0 and tblk == 0:
                        for nm, src in (("qkv", qkv), ("s", s_l), ("a", a_l),
                                        ("o", o_l)):
                            ff = stats.tile(list(src.shape), F32,
                                            tag=f"dbg{nm}", name=f"f{nm}")
                            nc.vector.tensor_copy(out=ff, in_=src)
                            nc.sync.dma_start(out=dbg[nm][:, :], in_=ff)
                    # transpose O back to C-major; one ACT eviction for all 4
                    # C-tiles (otcb columns [ob*cs + pos*128, +128))
                    pt = pmm.tile([128, 512], BF16, tag="pt")
                    for ob in range(CT):
                        nc.tensor.transpose(pt[:, ob * 128:(ob + 1) * 128],
                                            o_l[:, ob * 128:(ob + 1) * 128],
                                            ident)
                    pos = tblk % bpc
                    nc.scalar.copy(
                        out=bass.AP(tensor=otcb.tensor,
                                    offset=otcb.offset + pos * 128,
                                    ap=[list(otcb.ap[0]), [cs, CT], [1, 128]]),
                        in_=pt.rearrange("p (ob f) -> p ob f", ob=CT))

                    # proj + residual for chunk j once its OT columns exist.
                    # The residual is added in PSUM by an identity matmul over
                    # a bf16 copy of x (re-DMA'd per chunk so the x tiles die
                    # at norm time and the next batch's loads overlap);
                    # eviction adds proj_b on ACT.
                    if (tblk + 1) % bpc == 0:
                        j = tblk // bpc
                        ncs = slice(j * cs, (j + 1) * cs)
                        for c in range(CT):
                            xr = outsb.tile([128, cs], F32, tag="xr", name="xr")
                            nc.sync.dma_start(
                                out=xr, in_=x_d[b, c * 128:(c + 1) * 128, ncs])
                            xbf = outsb.tile([128, cs], BF16, tag="xbf",
                                             name="xbf")
                            nc.scalar.copy(out=xbf, in_=xr)
                            py = pmm2.tile([128, cs], F32, tag="py", name="py")
                            for ob in range(CT):
                                nc.tensor.matmul(
                                    py, pwT[ob][:, c * 128:(c + 1) * 128],
                                    otcb[:, ob * cs:(ob + 1) * cs],
                                    start=(ob == 0), stop=False)
                            nc.tensor.matmul(py, ident, xbf,
                                             start=False, stop=True)
                            ot = outsb.tile([128, cs], F32, tag="out",
                                            name="ot")
                            nc.scalar.activation(out=ot, in_=py,
                                                 func=ACTF.Identity,
                                                 bias=pbt[c], scale=1.0)
                            nc.sync.dma_start(
                                out=out_d[b, c * 128:(c + 1) * 128, ncs],
                                in_=ot)

                pend = None
                for blk in range(nblk):
                    if b + 1 < nb:
                        if blk == 4:
                            nxt["x"] = emit_xload(b + 1)
                        elif blk == 20:
                            nxt["st"] = emit_stats(b + 1, nxt["x"])
                        elif blk == 24:
                            nxt["h0"] = emit_norm(b + 1, 0, nxt["x"], *nxt["st"])
                        elif blk == 28:
                            nxt["h1"] = emit_norm(b + 1, 1, nxt["x"], *nxt["st"])
                    half = min(blk // hb, 1)
                    hslice = slice(blk * 128 - half * nh2, (blk + 1) * 128 - half * nh2)
                    pq = pqkv.tile([128, 3 * C], F32, tag="pq", name="pq")
                    for c in range(CT):
                        lhsT = ht[half][c][:, hslice]
                        for oc in range(OT3):
                            nc.tensor.matmul(
                                pq[:, oc * 512:(oc + 1) * 512], lhsT,
                                wqkvT[c][:, oc * 512:(oc + 1) * 512],
                                start=(c == 0), stop=(c == CT - 1))
                    qkv = qkvsb.tile([128, 3 * C], BF16, tag="qkv")
                    if qkbias is not None:
                        nc.vector.tensor_add(out=qkv, in0=pq, in1=qkbias)
                    else:
                        nc.scalar.copy(out=qkv, in_=pq)

                    q = qkv[:, 0:512]
                    k = qkv[:, 512:1024]

                    # u1 multiply runs on GPSIMD (TensorTensor — the only
                    # elementwise opcode GPSIMD codegen accepts) for most
                    # blocks; the d-trees all stay on DVE where the 2x bf16
                    # mode applies.
                    on_pool = (blk * NMULT) % nblk < NMULT
                    # logits: U1[(h,g,d)] = q[h,d] * k[g,d]; S = sum over d
                    u1 = upool.tile([128, NH * NH * HD], BF16, tag="u")
                    s_l = spool.tile([128, NH * NH], F32, tag="s")
                    u1v = u1.rearrange("p (a d) -> p a d", d=HD)
                    ueng = nc.gpsimd if on_pool else nc.vector
                    ueng.tensor_tensor(
                        out=u1.rearrange("p (h g d) -> p h g d", h=NH, g=NH),
                        in0=_bc(q, [(HD, NH), (0, NH), (1, HD)]),
                        in1=_bc(k, [(0, NH), (HD, NH), (1, HD)]),
                        op=ALU.mult)
                    w = HD
                    while w > 2:
                        nc.vector.tensor_tensor(
                            out=u1v[:, :, 0:w // 2], in0=u1v[:, :, 0:w // 2],
                            in1=u1v[:, :, w // 2:w], op=ALU.add)
                        w //= 2
                    nc.vector.tensor_tensor(
                        out=s_l.rearrange("p (a u) -> p a u", u=1),
                        in0=u1v[:, :, 0:1], in1=u1v[:, :, 1:2], op=ALU.add)
                    if pend is not None:
                        emit_tail(*pend)
                    pend = (blk, qkv, s_l)
                emit_tail(*pend)
                if b + 1 < nb:
                    if "x" not in nxt:
                        nxt["x"] = emit_xload(b + 1)
                    if "st" not in nxt:
                        nxt["st"] = emit_stats(b + 1, nxt["x"])
                    if "h0" not in nxt:
                        nxt["h0"] = emit_norm(b + 1, 0, nxt["x"], *nxt["st"])
                    if "h1" not in nxt:
                        nxt["h1"] = (emit_norm(b + 1, 1, nxt["x"], *nxt["st"])
                                     if nblk > 1 else nxt["h0"])
                    xt_cur, st_cur = nxt["x"], nxt["st"]
                    ht_cur = [nxt["h0"], nxt["h1"]]
    return nc


_CACHE = {}


def host_inputs(norm_w, norm_b, qkv_w, qkv_b, proj_w, proj_b):
    """Host-side weight preprocessing -> the kernel's shared input tensors."""
    bf = ml_dtypes.bfloat16
    # V-part column permutation: store V as [d*8+g] so the AV multiply reads
    # both operands at unit stride (DVE 2x mode).
    vperm = np.arange(3 * C)
    g_i, d_i = np.meshgrid(np.arange(NH), np.arange(HD), indexing="ij")
    vperm[2 * C:] = 2 * C + (d_i * NH + g_i).reshape(-1)   # old[g*64+d] -> new pos
    inv = np.empty_like(vperm)
    inv[vperm] = np.arange(3 * C)
    wq_p = qkv_w[inv]        # new column j holds old channel inv[j]
    qkvb_p = np.ascontiguousarray(qkv_b[inv])
    wqkvT = np.ascontiguousarray(wq_p.T).astype(bf)           # [C, 3C]
    pwT = np.ascontiguousarray(proj_w.T).astype(bf)           # [C(o), C(c)]
    ident = np.eye(128, dtype=np.float32).astype(bf)
    return dict(wqkvT=wqkvT, pwT=pwT,
                normw=np.asarray(norm_w, np.float32),
                normb=np.asarray(norm_b, np.float32),
                qkvb=qkvb_p, pbeff=np.asarray(proj_b, np.float32),
                ident=ident)


def kernel(x, norm_w, norm_b, qkv_w, qkv_b, proj_w, proj_b):
    x = np.asarray(x, np.float32)
    norm_w = np.asarray(norm_w, np.float32)
    norm_b = np.asarray(norm_b, np.float32)
    qkv_w = np.asarray(qkv_w, np.float32)
    qkv_b = np.asarray(qkv_b, np.float32)
    proj_w = np.asarray(proj_w, np.float32)
    proj_b = np.asarray(proj_b, np.float32)

    qk_bias = bool(np.any(qkv_b != 0))
    key = ("full", qk_bias)
    if key not in _CACHE:
        nc_new = build_kernel(qk_bias=qk_bias)
        _cap_sync_waits(nc_new)   # HW path only; CoreSim rejects bare NoOps
        _CACHE[key] = nc_new
    nc = _CACHE[key]

    shared = host_inputs(norm_w, norm_b, qkv_w, qkv_b, proj_w, proj_b)
    xs = x.reshape(B, C, N)
    in_maps = [dict(x=np.ascontiguousarray(xs[c * NB:(c + 1) * NB]), **shared)
               for c in range(NCORES)]
    res = run_bass_kernel_spmd(nc, in_maps, core_ids=list(range(NCORES)),
                               trace=bool(os.environ.get("KERNEL_TRACE")))
    global LAST_RES
    LAST_RES = res
    out = np.concatenate([res.results[c]["out"] for c in range(NCORES)], axis=0)
    return out.reshape(B, C, HH, WW).astype(np.float32)


LAST_RES = None



# revision 32
# speedup vs baseline: 1.1843x; 1.0370x over previous
"""Trainium2 Bass kernel for nn_AttentionBlock (GroupNorm + per-position
head-axis attention + proj + residual).

Sharding: data-parallel over batch B=16 -> 2 batches per core x 8 cores.
Each core runs an identical program on its x-shard [2, 512, 4096] plus
replicated (host-preprocessed) weights, and writes its out-shard.

Per-core pipeline:
  1. GroupNorm(32): bn_stats per partition over N; cross-partition group
     aggregation via two tiny SBUF->SBUF DMA gathers (DMA crosses
     partitions); normalize on ACT with per-partition scale/bias.
  2. QKV: out[n, o] via PE with h-block stationary -> QKV arrives N-major.
     h is normalized into two half-batch column groups (low half
     double-buffered) so consecutive batches overlap.
  3. Attention (N-major, per 128-position block): logits/AV as broadcast
     elementwise multiplies (bf16 unit-stride so the DVE 2x mode applies;
     the V weight columns are host-permuted to [d*8+g] for this), with the
     d- and g-reductions done as in-place halving add-trees (adds get 2x
     mode; InstTensorReduce would run 1x). Softmax skips max-subtraction
     (logits are O(1) by construction); Exp on ACT with the 1/8 scale
     folded in. The logits multiply runs on GPSIMD for 2/3 of blocks.
  4. O transposed back to C-major via PE transpose; proj matmul on PE
     consumes a 3-deep ring of per-chunk OT tiles; residual-add fused into
     the PSUM->SBUF eviction on DVE (x re-DMA'd per chunk); DMA out.

Host-side preprocessing: weight transposes + bf16 casts + V-column permute.
If qkv_b is nonzero the kernel emits bias adds (specialized at trace; the
benchmark uses zero biases).

_cap_sync_waits: this walrus build accepts only ONE sync wait per compute
instruction; Tile emits more. The pass hoists excess waits onto same-engine
InstNoOps inserted immediately before the offender.
"""

import os

import numpy as np
import ml_dtypes

import concourse.bass as bass
import concourse.mybir as mybir
import concourse.tile as tile
from concourse.bass_utils import run_bass_kernel_spmd

F32 = mybir.dt.float32
BF16 = mybir.dt.bfloat16

B, C, HH, WW = 16, 512, 64, 64
N = HH * WW            # 4096
NB = 2                 # batches per core
NCORES = 8
NH, HD = 8, 64         # heads, head dim
GROUPS = 32
GSIZE = C // GROUPS    # 16 channels per group
EPS = 1e-5
CT = C // 128          # 4 channel tiles
OT3 = 3 * C // 512     # 3 o-chunks of 512 in qkv
NBLK = N // 128        # 32 position blocks per batch

AX = mybir.AxisListType
ALU = mybir.AluOpType
ACTF = mybir.ActivationFunctionType

# d-columns of each logits multiply computed on DVE (rest on GPSIMD)
DSPLIT = 2


def _bc(t, dims):
    """AP over tile/AP `t` with explicit free [step,count] dims (elem units)."""
    return bass.AP(tensor=t.tensor, offset=t.offset,
                   ap=[list(t.ap[0])] + [list(d) for d in dims])


def _cap_sync_waits(nc):
    """Walrus (this neuronxcc) allows at most 2 sync waits per compute
    instruction and is stricter still for some DMA structs. Tile can emit
    more. Hoist the excess onto a same-engine InstNoOp inserted immediately
    before the offender — the waits still complete before it executes."""
    import bass_rust
    n = 0
    for f in nc.m.functions:
        for blk in f.blocks:
            il = blk.instructions
            i = 0
            while i < len(il):
                ins = il[i]
                si = getattr(ins, "sync_info", None)
                if si is not None and si.on_wait and len(si.on_wait) > 1:
                    waits = list(si.on_wait)
                    for w in waits[:-1]:
                        nop = mybir.InstNoOp(name=f"W-abs-{n}", ins=[], outs=[])
                        n += 1
                        nop.engine = ins.engine
                        nop.sync_info = bass_rust.SyncInfo(on_wait=[w],
                                                           on_update=[])
                        il.insert(i, nop)
                        i += 1
                    si.on_wait = waits[-1:]
                i += 1
    return n


def build_kernel(nb=NB, nblk=NBLK, qk_bias=False, debug=False):
    n = nblk * 128
    cs = min(512, n)       # proj/residual n-chunk
    nch = n // cs
    nc = bass.Bass()
    dbg = {}
    if debug:
        dbg["h"] = nc.dram_tensor("dbg_h", [C, n], F32, kind="ExternalOutput")
        dbg["qkv"] = nc.dram_tensor("dbg_qkv", [128, 3 * C], F32, kind="ExternalOutput")
        dbg["s"] = nc.dram_tensor("dbg_s", [128, NH * NH], F32, kind="ExternalOutput")
        dbg["a"] = nc.dram_tensor("dbg_a", [128, NH * NH], F32, kind="ExternalOutput")
        dbg["o"] = nc.dram_tensor("dbg_o", [128, C], F32, kind="ExternalOutput")
        dbg["otsb"] = nc.dram_tensor("dbg_otsb", [C, n], F32, kind="ExternalOutput")

    x_d = nc.dram_tensor("x", [nb, C, n], F32, kind="ExternalInput")
    wqkvT_d = nc.dram_tensor("wqkvT", [C, 3 * C], BF16, kind="ExternalInput")
    pwT_d = nc.dram_tensor("pwT", [C, C], BF16, kind="ExternalInput")
    normw_d = nc.dram_tensor("normw", [C], F32, kind="ExternalInput")
    normb_d = nc.dram_tensor("normb", [C], F32, kind="ExternalInput")
    qkvb_d = nc.dram_tensor("qkvb", [3 * C], F32, kind="ExternalInput")
    pbeff_d = nc.dram_tensor("pbeff", [C], F32, kind="ExternalInput")
    ident_d = nc.dram_tensor("ident", [128, 128], BF16, kind="ExternalInput")
    out_d = nc.dram_tensor("out", [nb, C, n], F32, kind="ExternalOutput")

    with tile.TileContext(nc) as tc:
        with (
            tc.tile_pool(name="consts", bufs=1) as consts,
            tc.tile_pool(name="xpool", bufs=1) as xpool,
            tc.tile_pool(name="hlo", bufs=2) as hlo,
            tc.tile_pool(name="hhi", bufs=1) as hhi,
            tc.tile_pool(name="otr", bufs=3) as otr,
            tc.tile_pool(name="stats", bufs=2) as stats,
            tc.tile_pool(name="scb", bufs=4) as scb,
            tc.tile_pool(name="qkvsb", bufs=3) as qkvsb,
            tc.tile_pool(name="upool", bufs=4) as upool,
            tc.tile_pool(name="spool", bufs=4) as spool,
            tc.tile_pool(name="opool", bufs=4) as opool,
            tc.tile_pool(name="outsb", bufs=2) as outsb,
            tc.tile_pool(name="pqkv", bufs=2, space="PSUM") as pqkv,   # 6 banks
            tc.tile_pool(name="pmm", bufs=1, space="PSUM") as pmm,     # 1 bank
            tc.tile_pool(name="pmm2", bufs=1, space="PSUM") as pmm2,   # 1 bank
        ):
            def emit_xload(b):
                xt = []
                for c in range(CT):
                    t = xpool.tile([128, n], F32, tag=f"x{c}")
                    nc.sync.dma_start(out=t, in_=x_d[b, c * 128:(c + 1) * 128, :])
                    xt.append(t)
                return xt

            # batch 0's x DMAs go first so GroupNorm stats start immediately;
            # the weight loads below overlap with them.
            xt_cur = emit_xload(0)

            # ---- constants / weights in SBUF ----
            wqkvT = []
            for c in range(CT):
                t = consts.tile([128, 3 * C], BF16, tag=f"wq{c}")
                nc.sync.dma_start(out=t, in_=wqkvT_d[c * 128:(c + 1) * 128, :])
                wqkvT.append(t)
            pwT = []
            for o in range(CT):
                t = consts.tile([128, C], BF16, tag=f"pw{o}")
                nc.sync.dma_start(out=t, in_=pwT_d[o * 128:(o + 1) * 128, :])
                pwT.append(t)
            ident = consts.tile([128, 128], BF16, tag="ident")
            nc.sync.dma_start(out=ident, in_=ident_d[:, :])
            nwt, nbt, pbt = [], [], []
            for c in range(CT):
                sl = slice(c * 128, (c + 1) * 128)
                t1 = consts.tile([128, 1], F32, tag=f"nw{c}")
                nc.sync.dma_start(out=t1, in_=normw_d[sl].rearrange("(p u) -> p u", u=1))
                nwt.append(t1)
                t2 = consts.tile([128, 1], F32, tag=f"nb{c}")
                nc.sync.dma_start(out=t2, in_=normb_d[sl].rearrange("(p u) -> p u", u=1))
                nbt.append(t2)
                t3 = consts.tile([128, 1], F32, tag=f"pb{c}")
                nc.sync.dma_start(out=t3, in_=pbeff_d[sl].rearrange("(p u) -> p u", u=1))
                pbt.append(t3)
            epst = consts.tile([1, 1], F32, tag="eps")
            nc.vector.memset(epst, 256.0 * EPS)
            qkbias = None
            if qk_bias:
                qkbias = consts.tile([128, 3 * C], F32, tag="qkb")
                nc.sync.dma_start(
                    out=qkbias,
                    in_=bass.AP(tensor=qkvb_d.ap().tensor, offset=0,
                                ap=[[0, 128], [1, 3 * C]]))

            hb = max(1, nblk // 2)          # blocks per half
            nh2 = hb * 128

            def emit_stats_p1(b, xt, c):
                    # ---------- GroupNorm stats, phase 1 (per c-tile) -------
                    # bn_stats + per-partition aggregation + the partition-
                    # gather DMA. Phase 2 is emitted later so the DMA
                    # round-trips of all four c-tiles overlap.
                    nsub = max(1, n // 512)
                    sd = nc.vector.BN_STATS_DIM
                    st = stats.tile([128, nsub, sd], F32, tag="bnst")
                    xv = xt[c].rearrange("p (s f) -> p s f", s=nsub)
                    for s in range(nsub):
                        nc.vector.bn_stats(out=st[:, s, :], in_=xv[:, s, :])
                    mv = stats.tile([128, nc.vector.BN_AGGR_DIM], F32, tag="bnmv")
                    nc.vector.bn_aggr(out=mv, in_=st)
                    # st2: col0 = mean, col1 = E[x^2] = var + mean^2
                    st2 = stats.tile([128, 2], F32, tag="st2")
                    nc.vector.tensor_copy(out=st2[:, 0:1], in_=mv[:, 0:1])
                    nc.vector.scalar_tensor_tensor(
                        out=st2[:, 1:2], in0=mv[:, 0:1], scalar=mv[:, 0:1],
                        in1=mv[:, 1:2], op0=ALU.mult, op1=ALU.add)
                    # gather all 128 partitions' stats onto one partition (DMA
                    # crosses partitions; avoids PE for the group aggregation)
                    stT = stats.tile([1, 256], F32, tag="stT")
                    nc.sync.dma_start(out=stT, in_=st2)
                    return stT

            def emit_stats_p2(b, stTs):
                    # ---------- GroupNorm stats, phase 2 ----------
                    scale_t, bias_t = [], []
                    for c in range(CT):
                        stT = stTs[c]
                        # per-group sums over the 16 channels: [1, 8, 2]
                        gsum = stats.tile([1, 16], F32, tag="gsum")
                        nc.vector.tensor_reduce(
                            out=gsum.rearrange("p (g c) -> p g c", g=8),
                            in_=_bc(stT, [(32, 8), (1, 2), (2, 16)]),
                            axis=AX.X, op=ALU.add)
                        gm = _bc(gsum, [(2, 8)])           # sum of means      [1,8]
                        ge = bass.AP(tensor=gsum.tensor, offset=gsum.offset + 1,
                                     ap=[list(gsum.ap[0])] + [[2, 8]])  # sum E[x^2]
                        m2 = stats.tile([1, 8], F32, tag="m2")
                        nc.vector.tensor_mul(m2, gm, gm)
                        # 256*var = 16*sum_ex2 - (sum_mean)^2
                        v256 = stats.tile([1, 8], F32, tag="v256")
                        nc.vector.scalar_tensor_tensor(
                            out=v256, in0=ge, scalar=16.0, in1=m2,
                            op0=ALU.mult, op1=ALU.subtract)
                        sg = stats.tile([1, 8], F32, tag="sg")
                        nc.scalar.activation(out=sg, in_=v256, func=ACTF.Sqrt,
                                             scale=1.0, bias=epst)   # sqrt(256(var+eps))
                        rg = stats.tile([1, 8], F32, tag="rg")
                        nc.vector.reciprocal(out=rg, in_=sg)          # rstd/16
                        # broadcast to 128 channel slots, interleaved (mean, rstd)
                        sb2 = stats.tile([1, 256], F32, tag="sb2")
                        nc.vector.tensor_scalar(
                            out=_bc(sb2, [(32, 8), (2, 16)]),
                            in0=_bc(gsum, [(2, 8), (0, 16)]), scalar1=1.0 / 16.0,
                            scalar2=None, op0=ALU.mult)
                        nc.vector.tensor_scalar(
                            out=bass.AP(tensor=sb2.tensor, offset=sb2.offset + 1,
                                        ap=[list(sb2.ap[0])] + [[32, 8], [2, 16]]),
                            in0=_bc(rg, [(1, 8), (0, 16)]), scalar1=16.0,
                            scalar2=None, op0=ALU.mult)
                        pb2 = stats.tile([128, 2], F32, tag="pb2")
                        nc.sync.dma_start(out=pb2, in_=sb2)
                        sc = scb.tile([128, 1], F32, tag="sc", name="sc")
                        bi = scb.tile([128, 1], F32, tag="bi", name="bi")
                        tmp = stats.tile([128, 1], F32, tag="tmp")
                        nc.vector.tensor_mul(sc, pb2[:, 1:2], nwt[c])
                        nc.vector.tensor_mul(tmp, pb2[:, 0:1], sc)
                        nc.vector.tensor_sub(bi, nbt[c], tmp)
                        scale_t.append(sc)
                        bias_t.append(bi)
                    return scale_t, bias_t

            def emit_norm(b, half, xt, scale_t, bias_t):
                hs = slice(half * nh2, min(n, (half + 1) * nh2))
                out = []
                for c in range(CT):
                    hp = hlo if half == 0 else hhi
                    t = hp.tile([128, nh2], BF16, tag=f"h{half}_{c}",
                                name=f"h{half}_{c}")
                    nc.scalar.activation(out=t, in_=xt[c][:, hs],
                                         func=ACTF.Identity,
                                         bias=bias_t[c], scale=scale_t[c])
                    out.append(t)
                    if debug and b == 0:
                        hf = stats.tile([128, nh2], F32, tag="dbgh", name="hf")
                        nc.vector.tensor_copy(out=hf, in_=t)
                        nc.sync.dma_start(
                            out=dbg["h"][c * 128:(c + 1) * 128, hs], in_=hf)
                return out

            def emit_stats(b, xt):
                return emit_stats_p2(b, [emit_stats_p1(b, xt, c)
                                         for c in range(CT)])

            st_cur = emit_stats(0, xt_cur)
            ht_cur = [emit_norm(0, 0, xt_cur, *st_cur), None]
            ht_cur[1] = emit_norm(0, 1, xt_cur, *st_cur) if nblk > 1 else ht_cur[0]
            for b in range(nb):
                xt, (scale_t, bias_t), ht = xt_cur, st_cur, ht_cur
                # ---------- per 128-position block ----------
                # The next batch's x load / stats / norm are emitted at fixed
                # points inside this loop so its head overlaps this batch's
                # attention tail (engine queues are in-order).
                nxt = {}
                bpc = cs // 128
                state = {"otcb": None}

                def emit_tail(tblk, qkv, s_l):
                    """Stage B of block tblk: softmax tail + AV + transpose +
                    proj. Emitted one block late so the ACT stream never
                    blocks the next block's qkv eviction behind exp()."""
                    v = qkv[:, 1024:1536]
                    if tblk % bpc == 0:
                        state["otcb"] = otr.tile([128, CT * cs], BF16,
                                                 tag="otr", name="otr")
                    otcb = state["otcb"]
                    # softmax over g: E = exp(S/8); logits bounded, no max-sub
                    e_l = spool.tile([128, NH * NH], BF16, tag="e")
                    nc.scalar.activation(out=e_l, in_=s_l, func=ACTF.Exp,
                                         scale=0.125)
                    d_l = spool.tile([128, NH], F32, tag="d")
                    nc.vector.tensor_reduce(
                        out=d_l, in_=e_l.rearrange("p (h g) -> p h g", g=NH),
                        axis=AX.X, op=ALU.add)
                    r_l = spool.tile([128, NH], F32, tag="r")
                    nc.vector.reciprocal(out=r_l, in_=d_l)
                    a_l = spool.tile([128, NH * NH], BF16, tag="a")
                    nc.vector.tensor_tensor(
                        out=a_l.rearrange("p (h g) -> p h g", g=NH),
                        in0=e_l.rearrange("p (h g) -> p h g", g=NH),
                        in1=_bc(r_l, [(1, NH), (0, NH)]),
                        op=ALU.mult)
                    # AV: U2[(h,d,g)] = A[h,g] * V'[d,g]; O = sum_g
                    # (V columns host-permuted to [d*8+g] so both reads are
                    # unit-stride innermost -> DVE 2x mode)
                    u2 = upool.tile([128, NH * HD * NH], BF16, tag="u")
                    nc.vector.tensor_tensor(
                        out=u2.rearrange("p (h d g) -> p h d g", h=NH, d=HD),
                        in0=_bc(a_l, [(NH, NH), (0, HD), (1, NH)]),
                        in1=_bc(v, [(0, NH), (NH, HD), (1, NH)]),
                        op=ALU.mult)
                    # O = sum_g via in-place halving adds (2x-mode eligible)
                    uv = u2.rearrange("p (a g) -> p a g", g=NH)
                    w = NH
                    while w > 2:
                        nc.vector.tensor_tensor(
                            out=uv[:, :, 0:w // 2], in0=uv[:, :, 0:w // 2],
                            in1=uv[:, :, w // 2:w], op=ALU.add)
                        w //= 2
                    o_l = opool.tile([128, C], BF16, tag="o")
                    nc.vector.tensor_tensor(
                        out=o_l.rearrange("p (a u) -> p a u", u=1),
                        in0=uv[:, :, 0:1], in1=uv[:, :, 1:2], op=ALU.add)
                    if debug and b == 0 and tblk == 0:
                        for nm, src in (("qkv", qkv), ("s", s_l), ("a", a_l),
                                        ("o", o_l)):
                            ff = stats.tile(list(src.shape), F32,
                                            tag=f"dbg{nm}", name=f"f{nm}")
                            nc.vector.tensor_copy(out=ff, in_=src)
                            nc.sync.dma_start(out=dbg[nm][:, :], in_=ff)
                    # transpose O back to C-major; one ACT eviction for all 4
                    # C-tiles (otcb columns [ob*cs + pos*128, +128))
                    pt = pmm.tile([128, 512], BF16, tag="pt")
                    for ob in range(CT):
                        nc.tensor.transpose(pt[:, ob * 128:(ob + 1) * 128],
                                            o_l[:, ob * 128:(ob + 1) * 128],
                                            ident)
                    pos = tblk % bpc
                    nc.scalar.copy(
                        out=bass.AP(tensor=otcb.tensor,
                                    offset=otcb.offset + pos * 128,
                                    ap=[list(otcb.ap[0]), [cs, CT], [1, 128]]),
                        in_=pt.rearrange("p (ob f) -> p ob f", ob=CT))

                    # proj + residual for chunk j once its OT columns exist.
                    # The residual is added in PSUM by an identity matmul over
                    # a bf16 copy of x (re-DMA'd per chunk so the x tiles die
                    # at norm time and the next batch's loads overlap);
                    # eviction adds proj_b on ACT.
                    if (tblk + 1) % bpc == 0:
                        j = tblk // bpc
                        ncs = slice(j * cs, (j + 1) * cs)
                        for c in range(CT):
                            xr = outsb.tile([128, cs], F32, tag="xr", name="xr")
                            nc.sync.dma_start(
                                out=xr, in_=x_d[b, c * 128:(c + 1) * 128, ncs])
                            xbf = outsb.tile([128, cs], BF16, tag="xbf",
                                             name="xbf")
                            nc.scalar.copy(out=xbf, in_=xr)
                            py = pmm2.tile([128, cs], F32, tag="py", name="py")
                            for ob in range(CT):
                                nc.tensor.matmul(
                                    py, pwT[ob][:, c * 128:(c + 1) * 128],
                                    otcb[:, ob * cs:(ob + 1) * cs],
                                    start=(ob == 0), stop=False)
                            nc.tensor.matmul(py, ident, xbf,
                                             start=False, stop=True)
                            ot = outsb.tile([128, cs], F32, tag="out",
                                            name="ot")
                            nc.scalar.activation(out=ot, in_=py,
                                                 func=ACTF.Identity,
                                                 bias=pbt[c], scale=1.0)
                            nc.sync.dma_start(
                                out=out_d[b, c * 128:(c + 1) * 128, ncs],
                                in_=ot)

                pend = None
                for blk in range(nblk):
                    if b + 1 < nb:
                        if blk == 4:
                            nxt["x"] = emit_xload(b + 1)
                        elif 16 <= blk < 16 + CT:
                            nxt.setdefault("stT", []).append(
                                emit_stats_p1(b + 1, nxt["x"], blk - 16))
                        elif blk == 21:
                            nxt["st"] = emit_stats_p2(b + 1, nxt["stT"])
                        elif blk == 24:
                            nxt["h0"] = emit_norm(b + 1, 0, nxt["x"], *nxt["st"])
                        elif blk == 28:
                            nxt["h1"] = emit_norm(b + 1, 1, nxt["x"], *nxt["st"])
                    half = min(blk // hb, 1)
                    hslice = slice(blk * 128 - half * nh2, (blk + 1) * 128 - half * nh2)
                    pq = pqkv.tile([128, 3 * C], F32, tag="pq", name="pq")
                    for c in range(CT):
                        lhsT = ht[half][c][:, hslice]
                        for oc in range(OT3):
                            nc.tensor.matmul(
                                pq[:, oc * 512:(oc + 1) * 512], lhsT,
                                wqkvT[c][:, oc * 512:(oc + 1) * 512],
                                start=(c == 0), stop=(c == CT - 1))
                    qkv = qkvsb.tile([128, 3 * C], BF16, tag="qkv")
                    if qkbias is not None:
                        nc.vector.tensor_add(out=qkv, in0=pq, in1=qkbias)
                    else:
                        nc.scalar.copy(out=qkv, in_=pq)

                    q = qkv[:, 0:512]
                    k = qkv[:, 512:1024]

                    # u1 multiply is split along d: the bulk runs on GPSIMD
                    # (TensorTensor — the only elementwise opcode GPSIMD
                    # codegen accepts), a small tail slice on DVE, sized so
                    # both engines pace at ~7.5us/block. The d-trees all stay
                    # on DVE where the 2x bf16 mode applies.
                    dp = HD - DSPLIT
                    # logits: U1[(h,g,d)] = q[h,d] * k[g,d]; S = sum over d
                    u1 = upool.tile([128, NH * NH * HD], BF16, tag="u")
                    s_l = spool.tile([128, NH * NH], F32, tag="s")
                    u1v = u1.rearrange("p (a d) -> p a d", d=HD)
                    u1hg = u1.rearrange("p (h g d) -> p h g d", h=NH, g=NH)
                    nc.gpsimd.tensor_tensor(
                        out=u1hg[:, :, :, 0:dp],
                        in0=_bc(q, [(HD, NH), (0, NH), (1, dp)]),
                        in1=_bc(k, [(0, NH), (HD, NH), (1, dp)]),
                        op=ALU.mult)
                    nc.vector.tensor_tensor(
                        out=u1hg[:, :, :, dp:HD],
                        in0=bass.AP(tensor=qkv.tensor,
                                    offset=qkv.offset + dp,
                                    ap=[list(qkv.ap[0]), [HD, NH], [0, NH],
                                        [1, DSPLIT]]),
                        in1=bass.AP(tensor=qkv.tensor,
                                    offset=qkv.offset + 512 + dp,
                                    ap=[list(qkv.ap[0]), [0, NH], [HD, NH],
                                        [1, DSPLIT]]),
                        op=ALU.mult)
                    w = HD
                    while w > 2:
                        nc.vector.tensor_tensor(
                            out=u1v[:, :, 0:w // 2], in0=u1v[:, :, 0:w // 2],
                            in1=u1v[:, :, w // 2:w], op=ALU.add)
                        w //= 2
                    nc.vector.tensor_tensor(
                        out=s_l.rearrange("p (a u) -> p a u", u=1),
                        in0=u1v[:, :, 0:1], in1=u1v[:, :, 1:2], op=ALU.add)
                    if pend is not None:
                        emit_tail(*pend)
                    pend = (blk, qkv, s_l)
                emit_tail(*pend)
                if b + 1 < nb:
                    if "x" not in nxt:
                        nxt["x"] = emit_xload(b + 1)
                    if "st" not in nxt:
                        nxt["st"] = emit_stats(b + 1, nxt["x"])
                    if "h0" not in nxt:
                        nxt["h0"] = emit_norm(b + 1, 0, nxt["x"], *nxt["st"])
                    if "h1" not in nxt:
                        nxt["h1"] = (emit_norm(b + 1, 1, nxt["x"], *nxt["st"])
                                     if nblk > 1 else nxt["h0"])
                    xt_cur, st_cur = nxt["x"], nxt["st"]
                    ht_cur = [nxt["h0"], nxt["h1"]]
    return nc


_CACHE = {}


def host_inputs(norm_w, norm_b, qkv_w, qkv_b, proj_w, proj_b):
    """Host-side weight preprocessing -> the kernel's shared input tensors."""
    bf = ml_dtypes.bfloat16
    # V-part column permutation: store V as [d*8+g] so the AV multiply reads
    # both operands at unit stride (DVE 2x mode).
    vperm = np.arange(3 * C)
    g_i, d_i = np.meshgrid(np.arange(NH), np.arange(HD), indexing="ij")
    vperm[2 * C:] = 2 * C + (d_i * NH + g_i).reshape(-1)   # old[g*64+d] -> new pos
    inv = np.empty_like(vperm)
    inv[vperm] = np.arange(3 * C)
    wq_p = qkv_w[inv]        # new column j holds old channel inv[j]
    qkvb_p = np.ascontiguousarray(qkv_b[inv])
    wqkvT = np.ascontiguousarray(wq_p.T).astype(bf)           # [C, 3C]
    pwT = np.ascontiguousarray(proj_w.T).astype(bf)           # [C(o), C(c)]
    ident = np.eye(128, dtype=np.float32).astype(bf)
    return dict(wqkvT=wqkvT, pwT=pwT,
                normw=np.asarray(norm_w, np.float32),
                normb=np.asarray(norm_b, np.float32),
                qkvb=qkvb_p, pbeff=np.asarray(proj_b, np.float32),
                ident=ident)


def kernel(x, norm_w, norm_b, qkv_w, qkv_b, proj_w, proj_b):
    x = np.asarray(x, np.float32)
    norm_w = np.asarray(norm_w, np.float32)
    norm_b = np.asarray(norm_b, np.float32)
    qkv_w = np.asarray(qkv_w, np.float32)
    qkv_b = np.asarray(qkv_b, np.float32)
    proj_w = np.asarray(proj_w, np.float32)
    proj_b = np.asarray(proj_b, np.float32)

    qk_bias = bool(np.any(qkv_b != 0))
    key = ("full", qk_bias)
    if key not in _CACHE:
        nc_new = build_kernel(qk_bias=qk_bias)
        _cap_sync_waits(nc_new)   # HW path only; CoreSim rejects bare NoOps
        _CACHE[key] = nc_new
    nc = _CACHE[key]

    shared = host_inputs(norm_w, norm_b, qkv_w, qkv_b, proj_w, proj_b)
    xs = x.reshape(B, C, N)
    in_maps = [dict(x=np.ascontiguousarray(xs[c * NB:(c + 1) * NB]), **shared)
               for c in range(NCORES)]
    res = run_bass_kernel_spmd(nc, in_maps, core_ids=list(range(NCORES)),
                               trace=bool(os.environ.get("KERNEL_TRACE")))
    global LAST_RES
    LAST_RES = res
    out = np.concatenate([res.results[c]["out"] for c in range(NCORES)], axis=0)
    return out.reshape(B, C, HH, WW).astype(np.float32)


LAST_RES = None



# revision 34
# speedup vs baseline: 1.2086x; 1.0205x over previous
"""Trainium2 Bass kernel for nn_AttentionBlock (GroupNorm + per-position
head-axis attention + proj + residual).

Sharding: data-parallel over batch B=16 -> 2 batches per core x 8 cores.
Each core runs an identical program on its x-shard [2, 512, 4096] plus
replicated (host-preprocessed) weights, and writes its out-shard.

Per-core pipeline:
  1. GroupNorm(32): bn_stats per partition over N; cross-partition group
     aggregation via two tiny SBUF->SBUF DMA gathers (DMA crosses
     partitions); normalize on ACT with per-partition scale/bias.
  2. QKV: out[n, o] via PE with h-block stationary -> QKV arrives N-major.
     h is normalized into two half-batch column groups (low half
     double-buffered) so consecutive batches overlap.
  3. Attention (N-major, per 128-position block): logits/AV as broadcast
     elementwise multiplies (bf16 unit-stride so the DVE 2x mode applies;
     the V weight columns are host-permuted to [d*8+g] for this), with the
     d- and g-reductions done as in-place halving add-trees (adds get 2x
     mode; InstTensorReduce would run 1x). Softmax skips max-subtraction
     (logits are O(1) by construction); Exp on ACT with the 1/8 scale
     folded in. The logits multiply runs on GPSIMD for 2/3 of blocks.
  4. O transposed back to C-major via PE transpose; proj matmul on PE
     consumes a 3-deep ring of per-chunk OT tiles; residual-add fused into
     the PSUM->SBUF eviction on DVE (x re-DMA'd per chunk); DMA out.

Host-side preprocessing: weight transposes + bf16 casts + V-column permute.
If qkv_b is nonzero the kernel emits bias adds (specialized at trace; the
benchmark uses zero biases).

_cap_sync_waits: this walrus build accepts only ONE sync wait per compute
instruction; Tile emits more. The pass hoists excess waits onto same-engine
InstNoOps inserted immediately before the offender.
"""

import os

import numpy as np
import ml_dtypes

import concourse.bass as bass
import concourse.mybir as mybir
import concourse.tile as tile
from concourse.bass_utils import run_bass_kernel_spmd

F32 = mybir.dt.float32
BF16 = mybir.dt.bfloat16

B, C, HH, WW = 16, 512, 64, 64
N = HH * WW            # 4096
NB = 2                 # batches per core
NCORES = 8
NH, HD = 8, 64         # heads, head dim
GROUPS = 32
GSIZE = C // GROUPS    # 16 channels per group
EPS = 1e-5
CT = C // 128          # 4 channel tiles
OT3 = 3 * C // 512     # 3 o-chunks of 512 in qkv
NBLK = N // 128        # 32 position blocks per batch

AX = mybir.AxisListType
ALU = mybir.AluOpType
ACTF = mybir.ActivationFunctionType

# d-columns of each logits multiply computed on DVE (rest on GPSIMD)
DSPLIT = 8
QKV_BUFS = 4
U_BUFS = 4
S_BUFS = 4


def _bc(t, dims):
    """AP over tile/AP `t` with explicit free [step,count] dims (elem units)."""
    return bass.AP(tensor=t.tensor, offset=t.offset,
                   ap=[list(t.ap[0])] + [list(d) for d in dims])


def _cap_sync_waits(nc):
    """Walrus (this neuronxcc) allows at most 2 sync waits per compute
    instruction and is stricter still for some DMA structs. Tile can emit
    more. Hoist the excess onto a same-engine InstNoOp inserted immediately
    before the offender — the waits still complete before it executes."""
    import bass_rust
    n = 0
    for f in nc.m.functions:
        for blk in f.blocks:
            il = blk.instructions
            i = 0
            while i < len(il):
                ins = il[i]
                si = getattr(ins, "sync_info", None)
                if si is not None and si.on_wait and len(si.on_wait) > 1:
                    waits = list(si.on_wait)
                    for w in waits[:-1]:
                        nop = mybir.InstNoOp(name=f"W-abs-{n}", ins=[], outs=[])
                        n += 1
                        nop.engine = ins.engine
                        nop.sync_info = bass_rust.SyncInfo(on_wait=[w],
                                                           on_update=[])
                        il.insert(i, nop)
                        i += 1
                    si.on_wait = waits[-1:]
                i += 1
    return n


def build_kernel(nb=NB, nblk=NBLK, qk_bias=False, debug=False):
    n = nblk * 128
    cs = min(512, n)       # proj/residual n-chunk
    nch = n // cs
    nc = bass.Bass()
    dbg = {}
    if debug:
        dbg["h"] = nc.dram_tensor("dbg_h", [C, n], F32, kind="ExternalOutput")
        dbg["qkv"] = nc.dram_tensor("dbg_qkv", [128, 3 * C], F32, kind="ExternalOutput")
        dbg["s"] = nc.dram_tensor("dbg_s", [128, NH * NH], F32, kind="ExternalOutput")
        dbg["a"] = nc.dram_tensor("dbg_a", [128, NH * NH], F32, kind="ExternalOutput")
        dbg["o"] = nc.dram_tensor("dbg_o", [128, C], F32, kind="ExternalOutput")
        dbg["otsb"] = nc.dram_tensor("dbg_otsb", [C, n], F32, kind="ExternalOutput")

    x_d = nc.dram_tensor("x", [nb, C, n], F32, kind="ExternalInput")
    wqkvT_d = nc.dram_tensor("wqkvT", [C, 3 * C], BF16, kind="ExternalInput")
    pwT_d = nc.dram_tensor("pwT", [C, C], BF16, kind="ExternalInput")
    normw_d = nc.dram_tensor("normw", [C], F32, kind="ExternalInput")
    normb_d = nc.dram_tensor("normb", [C], F32, kind="ExternalInput")
    qkvb_d = nc.dram_tensor("qkvb", [3 * C], F32, kind="ExternalInput")
    pbeff_d = nc.dram_tensor("pbeff", [C], F32, kind="ExternalInput")
    ident_d = nc.dram_tensor("ident", [128, 128], BF16, kind="ExternalInput")
    out_d = nc.dram_tensor("out", [nb, C, n], F32, kind="ExternalOutput")

    with tile.TileContext(nc) as tc:
        with (
            tc.tile_pool(name="consts", bufs=1) as consts,
            tc.tile_pool(name="xpool", bufs=1) as xpool,
            tc.tile_pool(name="hlo", bufs=2) as hlo,
            tc.tile_pool(name="hhi", bufs=1) as hhi,
            tc.tile_pool(name="otr", bufs=3) as otr,
            tc.tile_pool(name="stats", bufs=2) as stats,
            tc.tile_pool(name="scb", bufs=4) as scb,
            tc.tile_pool(name="qkvsb", bufs=QKV_BUFS) as qkvsb,
            tc.tile_pool(name="upool", bufs=U_BUFS) as upool,
            tc.tile_pool(name="spool", bufs=S_BUFS) as spool,
            tc.tile_pool(name="opool", bufs=4) as opool,
            tc.tile_pool(name="outsb", bufs=2) as outsb,
            tc.tile_pool(name="pqkv", bufs=2, space="PSUM") as pqkv,   # 6 banks
            tc.tile_pool(name="pmm", bufs=1, space="PSUM") as pmm,     # 1 bank
            tc.tile_pool(name="pmm2", bufs=1, space="PSUM") as pmm2,   # 1 bank
        ):
            def emit_xload(b):
                xt = []
                for c in range(CT):
                    t = xpool.tile([128, n], F32, tag=f"x{c}")
                    nc.sync.dma_start(out=t, in_=x_d[b, c * 128:(c + 1) * 128, :])
                    xt.append(t)
                return xt

            # batch 0's x DMAs go first so GroupNorm stats start immediately;
            # the weight loads below overlap with them.
            xt_cur = emit_xload(0)

            # ---- constants / weights in SBUF ----
            wqkvT = []
            for c in range(CT):
                t = consts.tile([128, 3 * C], BF16, tag=f"wq{c}")
                nc.sync.dma_start(out=t, in_=wqkvT_d[c * 128:(c + 1) * 128, :])
                wqkvT.append(t)
            pwT = []
            for o in range(CT):
                t = consts.tile([128, C], BF16, tag=f"pw{o}")
                nc.sync.dma_start(out=t, in_=pwT_d[o * 128:(o + 1) * 128, :])
                pwT.append(t)
            ident = consts.tile([128, 128], BF16, tag="ident")
            nc.sync.dma_start(out=ident, in_=ident_d[:, :])
            nwt, nbt, pbt = [], [], []
            for c in range(CT):
                sl = slice(c * 128, (c + 1) * 128)
                t1 = consts.tile([128, 1], F32, tag=f"nw{c}")
                nc.sync.dma_start(out=t1, in_=normw_d[sl].rearrange("(p u) -> p u", u=1))
                nwt.append(t1)
                t2 = consts.tile([128, 1], F32, tag=f"nb{c}")
                nc.sync.dma_start(out=t2, in_=normb_d[sl].rearrange("(p u) -> p u", u=1))
                nbt.append(t2)
                t3 = consts.tile([128, 1], F32, tag=f"pb{c}")
                nc.sync.dma_start(out=t3, in_=pbeff_d[sl].rearrange("(p u) -> p u", u=1))
                pbt.append(t3)
            epst = consts.tile([1, 1], F32, tag="eps")
            nc.vector.memset(epst, 256.0 * EPS)
            qkbias = None
            if qk_bias:
                qkbias = consts.tile([128, 3 * C], F32, tag="qkb")
                nc.sync.dma_start(
                    out=qkbias,
                    in_=bass.AP(tensor=qkvb_d.ap().tensor, offset=0,
                                ap=[[0, 128], [1, 3 * C]]))

            hb = max(1, nblk // 2)          # blocks per half
            nh2 = hb * 128

            def emit_stats_p1(b, xt, c):
                    # ---------- GroupNorm stats, phase 1 (per c-tile) -------
                    # bn_stats + per-partition aggregation + the partition-
                    # gather DMA. Phase 2 is emitted later so the DMA
                    # round-trips of all four c-tiles overlap.
                    nsub = max(1, n // 512)
                    sd = nc.vector.BN_STATS_DIM
                    st = stats.tile([128, nsub, sd], F32, tag="bnst")
                    xv = xt[c].rearrange("p (s f) -> p s f", s=nsub)
                    for s in range(nsub):
                        nc.vector.bn_stats(out=st[:, s, :], in_=xv[:, s, :])
                    mv = stats.tile([128, nc.vector.BN_AGGR_DIM], F32, tag="bnmv")
                    nc.vector.bn_aggr(out=mv, in_=st)
                    # st2: col0 = mean, col1 = E[x^2] = var + mean^2
                    st2 = stats.tile([128, 2], F32, tag="st2")
                    nc.vector.tensor_copy(out=st2[:, 0:1], in_=mv[:, 0:1])
                    nc.vector.scalar_tensor_tensor(
                        out=st2[:, 1:2], in0=mv[:, 0:1], scalar=mv[:, 0:1],
                        in1=mv[:, 1:2], op0=ALU.mult, op1=ALU.add)
                    # gather all 128 partitions' stats onto one partition (DMA
                    # crosses partitions; avoids PE for the group aggregation)
                    stT = stats.tile([1, 256], F32, tag="stT")
                    nc.sync.dma_start(out=stT, in_=st2)
                    return stT

            def emit_stats_p2(b, stTs):
                    # ---------- GroupNorm stats, phase 2 ----------
                    scale_t, bias_t = [], []
                    for c in range(CT):
                        stT = stTs[c]
                        # per-group sums over the 16 channels: [1, 8, 2]
                        gsum = stats.tile([1, 16], F32, tag="gsum")
                        nc.vector.tensor_reduce(
                            out=gsum.rearrange("p (g c) -> p g c", g=8),
                            in_=_bc(stT, [(32, 8), (1, 2), (2, 16)]),
                            axis=AX.X, op=ALU.add)
                        gm = _bc(gsum, [(2, 8)])           # sum of means      [1,8]
                        ge = bass.AP(tensor=gsum.tensor, offset=gsum.offset + 1,
                                     ap=[list(gsum.ap[0])] + [[2, 8]])  # sum E[x^2]
                        m2 = stats.tile([1, 8], F32, tag="m2")
                        nc.vector.tensor_mul(m2, gm, gm)
                        # 256*var = 16*sum_ex2 - (sum_mean)^2
                        v256 = stats.tile([1, 8], F32, tag="v256")
                        nc.vector.scalar_tensor_tensor(
                            out=v256, in0=ge, scalar=16.0, in1=m2,
                            op0=ALU.mult, op1=ALU.subtract)
                        sg = stats.tile([1, 8], F32, tag="sg")
                        nc.scalar.activation(out=sg, in_=v256, func=ACTF.Sqrt,
                                             scale=1.0, bias=epst)   # sqrt(256(var+eps))
                        rg = stats.tile([1, 8], F32, tag="rg")
                        nc.vector.reciprocal(out=rg, in_=sg)          # rstd/16
                        # broadcast to 128 channel slots, interleaved (mean, rstd)
                        sb2 = stats.tile([1, 256], F32, tag="sb2")
                        nc.vector.tensor_scalar(
                            out=_bc(sb2, [(32, 8), (2, 16)]),
                            in0=_bc(gsum, [(2, 8), (0, 16)]), scalar1=1.0 / 16.0,
                            scalar2=None, op0=ALU.mult)
                        nc.vector.tensor_scalar(
                            out=bass.AP(tensor=sb2.tensor, offset=sb2.offset + 1,
                                        ap=[list(sb2.ap[0])] + [[32, 8], [2, 16]]),
                            in0=_bc(rg, [(1, 8), (0, 16)]), scalar1=16.0,
                            scalar2=None, op0=ALU.mult)
                        pb2 = stats.tile([128, 2], F32, tag="pb2")
                        nc.sync.dma_start(out=pb2, in_=sb2)
                        sc = scb.tile([128, 1], F32, tag="sc", name="sc")
                        bi = scb.tile([128, 1], F32, tag="bi", name="bi")
                        tmp = stats.tile([128, 1], F32, tag="tmp")
                        nc.vector.tensor_mul(sc, pb2[:, 1:2], nwt[c])
                        nc.vector.tensor_mul(tmp, pb2[:, 0:1], sc)
                        nc.vector.tensor_sub(bi, nbt[c], tmp)
                        scale_t.append(sc)
                        bias_t.append(bi)
                    return scale_t, bias_t

            def emit_norm(b, half, xt, scale_t, bias_t):
                hs = slice(half * nh2, min(n, (half + 1) * nh2))
                out = []
                for c in range(CT):
                    hp = hlo if half == 0 else hhi
                    t = hp.tile([128, nh2], BF16, tag=f"h{half}_{c}",
                                name=f"h{half}_{c}")
                    nc.scalar.activation(out=t, in_=xt[c][:, hs],
                                         func=ACTF.Identity,
                                         bias=bias_t[c], scale=scale_t[c])
                    out.append(t)
                    if debug and b == 0:
                        hf = stats.tile([128, nh2], F32, tag="dbgh", name="hf")
                        nc.vector.tensor_copy(out=hf, in_=t)
                        nc.sync.dma_start(
                            out=dbg["h"][c * 128:(c + 1) * 128, hs], in_=hf)
                return out

            def emit_stats(b, xt):
                return emit_stats_p2(b, [emit_stats_p1(b, xt, c)
                                         for c in range(CT)])

            st_cur = emit_stats(0, xt_cur)
            ht_cur = [emit_norm(0, 0, xt_cur, *st_cur), None]
            ht_cur[1] = emit_norm(0, 1, xt_cur, *st_cur) if nblk > 1 else ht_cur[0]
            for b in range(nb):
                xt, (scale_t, bias_t), ht = xt_cur, st_cur, ht_cur
                # ---------- per 128-position block ----------
                # The next batch's x load / stats / norm are emitted at fixed
                # points inside this loop so its head overlaps this batch's
                # attention tail (engine queues are in-order).
                nxt = {}
                bpc = cs // 128
                state = {"otcb": None}

                def emit_tail(tblk, qkv, s_l):
                    """Stage B of block tblk: softmax tail + AV + transpose +
                    proj. Emitted one block late so the ACT stream never
                    blocks the next block's qkv eviction behind exp()."""
                    v = qkv[:, 1024:1536]
                    if tblk % bpc == 0:
                        state["otcb"] = otr.tile([128, CT * cs], BF16,
                                                 tag="otr", name="otr")
                    otcb = state["otcb"]
                    # softmax over g: E = exp(S/8); logits bounded, no max-sub
                    e_l = spool.tile([128, NH * NH], BF16, tag="e")
                    nc.scalar.activation(out=e_l, in_=s_l, func=ACTF.Exp,
                                         scale=0.125)
                    d_l = spool.tile([128, NH], F32, tag="d")
                    nc.vector.tensor_reduce(
                        out=d_l, in_=e_l.rearrange("p (h g) -> p h g", g=NH),
                        axis=AX.X, op=ALU.add)
                    r_l = spool.tile([128, NH], F32, tag="r")
                    nc.vector.reciprocal(out=r_l, in_=d_l)
                    a_l = spool.tile([128, NH * NH], BF16, tag="a")
                    nc.vector.tensor_tensor(
                        out=a_l.rearrange("p (h g) -> p h g", g=NH),
                        in0=e_l.rearrange("p (h g) -> p h g", g=NH),
                        in1=_bc(r_l, [(1, NH), (0, NH)]),
                        op=ALU.mult)
                    # AV: U2[(h,d,g)] = A[h,g] * V'[d,g]; O = sum_g
                    # (V columns host-permuted to [d*8+g] so both reads are
                    # unit-stride innermost -> DVE 2x mode)
                    u2 = upool.tile([128, NH * HD * NH], BF16, tag="u")
                    nc.vector.tensor_tensor(
                        out=u2.rearrange("p (h d g) -> p h d g", h=NH, d=HD),
                        in0=_bc(a_l, [(NH, NH), (0, HD), (1, NH)]),
                        in1=_bc(v, [(0, NH), (NH, HD), (1, NH)]),
                        op=ALU.mult)
                    # O = sum_g via in-place halving adds (2x-mode eligible)
                    uv = u2.rearrange("p (a g) -> p a g", g=NH)
                    w = NH
                    while w > 2:
                        nc.vector.tensor_tensor(
                            out=uv[:, :, 0:w // 2], in0=uv[:, :, 0:w // 2],
                            in1=uv[:, :, w // 2:w], op=ALU.add)
                        w //= 2
                    o_l = opool.tile([128, C], BF16, tag="o")
                    nc.vector.tensor_tensor(
                        out=o_l.rearrange("p (a u) -> p a u", u=1),
                        in0=uv[:, :, 0:1], in1=uv[:, :, 1:2], op=ALU.add)
                    if debug and b == 0 and tblk == 0:
                        for nm, src in (("qkv", qkv), ("s", s_l), ("a", a_l),
                                        ("o", o_l)):
                            ff = stats.tile(list(src.shape), F32,
                                            tag=f"dbg{nm}", name=f"f{nm}")
                            nc.vector.tensor_copy(out=ff, in_=src)
                            nc.sync.dma_start(out=dbg[nm][:, :], in_=ff)
                    # transpose O back to C-major; one ACT eviction for all 4
                    # C-tiles (otcb columns [ob*cs + pos*128, +128))
                    pt = pmm.tile([128, 512], BF16, tag="pt")
                    for ob in range(CT):
                        nc.tensor.transpose(pt[:, ob * 128:(ob + 1) * 128],
                                            o_l[:, ob * 128:(ob + 1) * 128],
                                            ident)
                    pos = tblk % bpc
                    nc.scalar.copy(
                        out=bass.AP(tensor=otcb.tensor,
                                    offset=otcb.offset + pos * 128,
                                    ap=[list(otcb.ap[0]), [cs, CT], [1, 128]]),
                        in_=pt.rearrange("p (ob f) -> p ob f", ob=CT))

                    # proj + residual for chunk j once its OT columns exist.
                    # The residual is added in PSUM by an identity matmul over
                    # a bf16 copy of x (re-DMA'd per chunk so the x tiles die
                    # at norm time and the next batch's loads overlap);
                    # eviction adds proj_b on ACT.
                    if (tblk + 1) % bpc == 0:
                        j = tblk // bpc
                        ncs = slice(j * cs, (j + 1) * cs)
                        for c in range(CT):
                            xr = outsb.tile([128, cs], F32, tag="xr", name="xr")
                            nc.sync.dma_start(
                                out=xr, in_=x_d[b, c * 128:(c + 1) * 128, ncs])
                            xbf = outsb.tile([128, cs], BF16, tag="xbf",
                                             name="xbf")
                            nc.scalar.copy(out=xbf, in_=xr)
                            py = pmm2.tile([128, cs], F32, tag="py", name="py")
                            for ob in range(CT):
                                nc.tensor.matmul(
                                    py, pwT[ob][:, c * 128:(c + 1) * 128],
                                    otcb[:, ob * cs:(ob + 1) * cs],
                                    start=(ob == 0), stop=False)
                            nc.tensor.matmul(py, ident, xbf,
                                             start=False, stop=True)
                            ot = outsb.tile([128, cs], F32, tag="out",
                                            name="ot")
                            nc.scalar.activation(out=ot, in_=py,
                                                 func=ACTF.Identity,
                                                 bias=pbt[c], scale=1.0)
                            nc.sync.dma_start(
                                out=out_d[b, c * 128:(c + 1) * 128, ncs],
                                in_=ot)

                pend = None
                for blk in range(nblk):
                    if b + 1 < nb:
                        if blk == 4:
                            nxt["x"] = emit_xload(b + 1)
                        elif 16 <= blk < 16 + CT:
                            nxt.setdefault("stT", []).append(
                                emit_stats_p1(b + 1, nxt["x"], blk - 16))
                        elif blk == 21:
                            nxt["st"] = emit_stats_p2(b + 1, nxt["stT"])
                        elif blk == 24:
                            nxt["h0"] = emit_norm(b + 1, 0, nxt["x"], *nxt["st"])
                        elif blk == 28:
                            nxt["h1"] = emit_norm(b + 1, 1, nxt["x"], *nxt["st"])
                    half = min(blk // hb, 1)
                    hslice = slice(blk * 128 - half * nh2, (blk + 1) * 128 - half * nh2)
                    pq = pqkv.tile([128, 3 * C], F32, tag="pq", name="pq")
                    for c in range(CT):
                        lhsT = ht[half][c][:, hslice]
                        for oc in range(OT3):
                            nc.tensor.matmul(
                                pq[:, oc * 512:(oc + 1) * 512], lhsT,
                                wqkvT[c][:, oc * 512:(oc + 1) * 512],
                                start=(c == 0), stop=(c == CT - 1))
                    qkv = qkvsb.tile([128, 3 * C], BF16, tag="qkv")
                    if qkbias is not None:
                        nc.vector.tensor_add(out=qkv, in0=pq, in1=qkbias)
                    else:
                        nc.scalar.copy(out=qkv, in_=pq)

                    q = qkv[:, 0:512]
                    k = qkv[:, 512:1024]

                    # u1 multiply is split along d: the bulk runs on GPSIMD
                    # (TensorTensor — the only elementwise opcode GPSIMD
                    # codegen accepts), a small tail slice on DVE, sized so
                    # both engines pace at ~7.5us/block. The d-trees all stay
                    # on DVE where the 2x bf16 mode applies.
                    dp = HD - DSPLIT
                    # logits: U1[(h,g,d)] = q[h,d] * k[g,d]; S = sum over d
                    u1 = upool.tile([128, NH * NH * HD], BF16, tag="u")
                    s_l = spool.tile([128, NH * NH], F32, tag="s")
                    u1v = u1.rearrange("p (a d) -> p a d", d=HD)
                    u1hg = u1.rearrange("p (h g d) -> p h g d", h=NH, g=NH)
                    nc.gpsimd.tensor_tensor(
                        out=u1hg[:, :, :, 0:dp],
                        in0=_bc(q, [(HD, NH), (0, NH), (1, dp)]),
                        in1=_bc(k, [(0, NH), (HD, NH), (1, dp)]),
                        op=ALU.mult)
                    nc.vector.tensor_tensor(
                        out=u1hg[:, :, :, dp:HD],
                        in0=bass.AP(tensor=qkv.tensor,
                                    offset=qkv.offset + dp,
                                    ap=[list(qkv.ap[0]), [HD, NH], [0, NH],
                                        [1, DSPLIT]]),
                        in1=bass.AP(tensor=qkv.tensor,
                                    offset=qkv.offset + 512 + dp,
                                    ap=[list(qkv.ap[0]), [0, NH], [HD, NH],
                                        [1, DSPLIT]]),
                        op=ALU.mult)
                    w = HD
                    while w > 2:
                        nc.vector.tensor_tensor(
                            out=u1v[:, :, 0:w // 2], in0=u1v[:, :, 0:w // 2],
                            in1=u1v[:, :, w // 2:w], op=ALU.add)
                        w //= 2
                    nc.vector.tensor_tensor(
                        out=s_l.rearrange("p (a u) -> p a u", u=1),
                        in0=u1v[:, :, 0:1], in1=u1v[:, :, 1:2], op=ALU.add)
                    if pend is not None:
                        emit_tail(*pend)
                    pend = (blk, qkv, s_l)
                emit_tail(*pend)
                if b + 1 < nb:
                    if "x" not in nxt:
                        nxt["x"] = emit_xload(b + 1)
                    if "st" not in nxt:
                        nxt["st"] = emit_stats(b + 1, nxt["x"])
                    if "h0" not in nxt:
                        nxt["h0"] = emit_norm(b + 1, 0, nxt["x"], *nxt["st"])
                    if "h1" not in nxt:
                        nxt["h1"] = (emit_norm(b + 1, 1, nxt["x"], *nxt["st"])
                                     if nblk > 1 else nxt["h0"])
                    xt_cur, st_cur = nxt["x"], nxt["st"]
                    ht_cur = [nxt["h0"], nxt["h1"]]
    return nc


_CACHE = {}


def host_inputs(norm_w, norm_b, qkv_w, qkv_b, proj_w, proj_b):
    """Host-side weight preprocessing -> the kernel's shared input tensors."""
    bf = ml_dtypes.bfloat16
    # V-part column permutation: store V as [d*8+g] so the AV multiply reads
    # both operands at unit stride (DVE 2x mode).
    vperm = np.arange(3 * C)
    g_i, d_i = np.meshgrid(np.arange(NH), np.arange(HD), indexing="ij")
    vperm[2 * C:] = 2 * C + (d_i * NH + g_i).reshape(-1)   # old[g*64+d] -> new pos
    inv = np.empty_like(vperm)
    inv[vperm] = np.arange(3 * C)
    wq_p = qkv_w[inv]        # new column j holds old channel inv[j]
    qkvb_p = np.ascontiguousarray(qkv_b[inv])
    wqkvT = np.ascontiguousarray(wq_p.T).astype(bf)           # [C, 3C]
    pwT = np.ascontiguousarray(proj_w.T).astype(bf)           # [C(o), C(c)]
    ident = np.eye(128, dtype=np.float32).astype(bf)
    return dict(wqkvT=wqkvT, pwT=pwT,
                normw=np.asarray(norm_w, np.float32),
                normb=np.asarray(norm_b, np.float32),
                qkvb=qkvb_p, pbeff=np.asarray(proj_b, np.float32),
                ident=ident)


def kernel(x, norm_w, norm_b, qkv_w, qkv_b, proj_w, proj_b):
    x = np.asarray(x, np.float32)
    norm_w = np.asarray(norm_w, np.float32)
    norm_b = np.asarray(norm_b, np.float32)
    qkv_w = np.asarray(qkv_w, np.float32)
    qkv_b = np.asarray(qkv_b, np.float32)
    proj_w = np.asarray(proj_w, np.float32)
    proj_b = np.asarray(proj_b, np.float32)

    qk_bias = bool(np.any(qkv_b != 0))
    key = ("full", qk_bias)
    if key not in _CACHE:
        nc_new = build_kernel(qk_bias=qk_bias)
        _cap_sync_waits(nc_new)   # HW path only; CoreSim rejects bare NoOps
        _CACHE[key] = nc_new
    nc = _CACHE[key]

    shared = host_inputs(norm_w, norm_b, qkv_w, qkv_b, proj_w, proj_b)
    xs = x.reshape(B, C, N)
    in_maps = [dict(x=np.ascontiguousarray(xs[c * NB:(c + 1) * NB]), **shared)
               for c in range(NCORES)]
    res = run_bass_kernel_spmd(nc, in_maps, core_ids=list(range(NCORES)),
                               trace=bool(os.environ.get("KERNEL_TRACE")))
    global LAST_RES
    LAST_RES = res
    out = np.concatenate([res.results[c]["out"] for c in range(NCORES)], axis=0)
    return out.reshape(B, C, HH, WW).astype(np.float32)


LAST_RES = None



# revision 39
# speedup vs baseline: 1.2701x; 1.0509x over previous
"""Trainium2 Bass kernel for nn_AttentionBlock (GroupNorm + per-position
head-axis attention + proj + residual).

Sharding: data-parallel over batch B=16 -> 2 batches per core x 8 cores.
Each core runs an identical program on its x-shard [2, 512, 4096] plus
replicated (host-preprocessed) weights, and writes its out-shard.

Per-core pipeline:
  1. GroupNorm(32): bn_stats per partition over N; cross-partition group
     aggregation via two tiny SBUF->SBUF DMA gathers (DMA crosses
     partitions); normalize on ACT with per-partition scale/bias.
  2. QKV: out[n, o] via PE with h-block stationary -> QKV arrives N-major.
     h is normalized into two half-batch column groups (low half
     double-buffered) so consecutive batches overlap.
  3. Attention (N-major, per 128-position block): logits/AV as broadcast
     elementwise multiplies (bf16 unit-stride so the DVE 2x mode applies;
     the V weight columns are host-permuted to [d*8+g] for this), with the
     d- and g-reductions done as in-place halving add-trees (adds get 2x
     mode; InstTensorReduce would run 1x). Softmax skips max-subtraction
     (logits are O(1) by construction); Exp on ACT with the 1/8 scale
     folded in. The logits multiply runs on GPSIMD for 2/3 of blocks.
  4. O transposed back to C-major via PE transpose; proj matmul on PE
     consumes a 3-deep ring of per-chunk OT tiles; residual-add fused into
     the PSUM->SBUF eviction on DVE (x re-DMA'd per chunk); DMA out.

Host-side preprocessing: weight transposes + bf16 casts + V-column permute.
If qkv_b is nonzero the kernel emits bias adds (specialized at trace; the
benchmark uses zero biases).

_cap_sync_waits: this walrus build accepts only ONE sync wait per compute
instruction; Tile emits more. The pass hoists excess waits onto same-engine
InstNoOps inserted immediately before the offender.
"""

import os

import numpy as np
import ml_dtypes

import concourse.bass as bass
import concourse.mybir as mybir
import concourse.tile as tile
from concourse.bass_utils import run_bass_kernel_spmd

F32 = mybir.dt.float32
BF16 = mybir.dt.bfloat16

B, C, HH, WW = 16, 512, 64, 64
N = HH * WW            # 4096
NB = 2                 # batches per core
NCORES = 8
NH, HD = 8, 64         # heads, head dim
GROUPS = 32
GSIZE = C // GROUPS    # 16 channels per group
EPS = 1e-5
CT = C // 128          # 4 channel tiles
OT3 = 3 * C // 512     # 3 o-chunks of 512 in qkv
NBLK = N // 128        # 32 position blocks per batch

AX = mybir.AxisListType
ALU = mybir.AluOpType
ACTF = mybir.ActivationFunctionType

# d-columns of each logits multiply computed on DVE (rest on GPSIMD)
DSPLIT = 8
QKV_BUFS = 4
U_BUFS = 4
S_BUFS = 4


def _bc(t, dims):
    """AP over tile/AP `t` with explicit free [step,count] dims (elem units)."""
    return bass.AP(tensor=t.tensor, offset=t.offset,
                   ap=[list(t.ap[0])] + [list(d) for d in dims])


def _cap_sync_waits(nc):
    """Walrus (this neuronxcc) allows at most 2 sync waits per compute
    instruction and is stricter still for some DMA structs. Tile can emit
    more. Hoist the excess onto a same-engine InstNoOp inserted immediately
    before the offender — the waits still complete before it executes."""
    import bass_rust
    n = 0
    for f in nc.m.functions:
        for blk in f.blocks:
            il = blk.instructions
            i = 0
            while i < len(il):
                ins = il[i]
                si = getattr(ins, "sync_info", None)
                if si is not None and si.on_wait and len(si.on_wait) > 1:
                    waits = list(si.on_wait)
                    for w in waits[:-1]:
                        nop = mybir.InstNoOp(name=f"W-abs-{n}", ins=[], outs=[])
                        n += 1
                        nop.engine = ins.engine
                        nop.sync_info = bass_rust.SyncInfo(on_wait=[w],
                                                           on_update=[])
                        il.insert(i, nop)
                        i += 1
                    si.on_wait = waits[-1:]
                i += 1
    return n


def build_kernel(nb=NB, nblk=NBLK, qk_bias=False, debug=False):
    n = nblk * 128
    cs = min(512, n)       # proj/residual n-chunk
    nch = n // cs
    nc = bass.Bass()
    dbg = {}
    if debug:
        dbg["h"] = nc.dram_tensor("dbg_h", [C, n], F32, kind="ExternalOutput")
        dbg["qkv"] = nc.dram_tensor("dbg_qkv", [128, 3 * C], F32, kind="ExternalOutput")
        dbg["s"] = nc.dram_tensor("dbg_s", [128, NH * NH], F32, kind="ExternalOutput")
        dbg["a"] = nc.dram_tensor("dbg_a", [128, NH * NH], F32, kind="ExternalOutput")
        dbg["o"] = nc.dram_tensor("dbg_o", [128, C], F32, kind="ExternalOutput")
        dbg["otsb"] = nc.dram_tensor("dbg_otsb", [C, n], F32, kind="ExternalOutput")

    x_d = nc.dram_tensor("x", [nb, C, n], F32, kind="ExternalInput")
    wqkvT_d = nc.dram_tensor("wqkvT", [C, 3 * C], BF16, kind="ExternalInput")
    pwT_d = nc.dram_tensor("pwT", [C, C], BF16, kind="ExternalInput")
    normw_d = nc.dram_tensor("normw", [C], F32, kind="ExternalInput")
    normb_d = nc.dram_tensor("normb", [C], F32, kind="ExternalInput")
    qkvb_d = nc.dram_tensor("qkvb", [3 * C], F32, kind="ExternalInput")
    pbeff_d = nc.dram_tensor("pbeff", [C], F32, kind="ExternalInput")
    ident_d = nc.dram_tensor("ident", [128, 128], BF16, kind="ExternalInput")
    out_d = nc.dram_tensor("out", [nb, C, n], F32, kind="ExternalOutput")

    with tile.TileContext(nc) as tc:
        with (
            tc.tile_pool(name="consts", bufs=1) as consts,
            tc.tile_pool(name="xpool", bufs=1) as xpool,
            tc.tile_pool(name="hlo", bufs=2) as hlo,
            tc.tile_pool(name="hhi", bufs=1) as hhi,
            tc.tile_pool(name="otr", bufs=3) as otr,
            tc.tile_pool(name="stats", bufs=2) as stats,
            tc.tile_pool(name="scb", bufs=4) as scb,
            tc.tile_pool(name="qkvsb", bufs=QKV_BUFS) as qkvsb,
            tc.tile_pool(name="upool", bufs=U_BUFS) as upool,
            tc.tile_pool(name="spool", bufs=S_BUFS) as spool,
            tc.tile_pool(name="opool", bufs=4) as opool,
            tc.tile_pool(name="outsb", bufs=2) as outsb,
            tc.tile_pool(name="pqkv", bufs=2, space="PSUM") as pqkv,   # 6 banks
            tc.tile_pool(name="pmm", bufs=1, space="PSUM") as pmm,     # 1 bank
            tc.tile_pool(name="pmm2", bufs=1, space="PSUM") as pmm2,   # 1 bank
        ):
            def emit_xload(b):
                xt = []
                for c in range(CT):
                    t = xpool.tile([128, n], F32, tag=f"x{c}")
                    nc.sync.dma_start(out=t, in_=x_d[b, c * 128:(c + 1) * 128, :])
                    xt.append(t)
                return xt

            # batch 0's x DMAs go first so GroupNorm stats start immediately;
            # the weight loads below overlap with them.
            xt_cur = emit_xload(0)

            # ---- constants / weights in SBUF ----
            wqkvT = []
            for c in range(CT):
                t = consts.tile([128, 3 * C], BF16, tag=f"wq{c}")
                nc.sync.dma_start(out=t, in_=wqkvT_d[c * 128:(c + 1) * 128, :])
                wqkvT.append(t)
            pwT = []
            for o in range(CT):
                t = consts.tile([128, C], BF16, tag=f"pw{o}")
                nc.sync.dma_start(out=t, in_=pwT_d[o * 128:(o + 1) * 128, :])
                pwT.append(t)
            ident = consts.tile([128, 128], BF16, tag="ident")
            nc.sync.dma_start(out=ident, in_=ident_d[:, :])
            nwt, nbt, pbt = [], [], []
            for c in range(CT):
                sl = slice(c * 128, (c + 1) * 128)
                t1 = consts.tile([128, 1], F32, tag=f"nw{c}")
                nc.sync.dma_start(out=t1, in_=normw_d[sl].rearrange("(p u) -> p u", u=1))
                nwt.append(t1)
                t2 = consts.tile([128, 1], F32, tag=f"nb{c}")
                nc.sync.dma_start(out=t2, in_=normb_d[sl].rearrange("(p u) -> p u", u=1))
                nbt.append(t2)
                t3 = consts.tile([128, 1], F32, tag=f"pb{c}")
                nc.sync.dma_start(out=t3, in_=pbeff_d[sl].rearrange("(p u) -> p u", u=1))
                pbt.append(t3)
            epst = consts.tile([1, 1], F32, tag="eps")
            nc.vector.memset(epst, 256.0 * EPS)
            qkbias = None
            if qk_bias:
                qkbias = consts.tile([128, 3 * C], F32, tag="qkb")
                nc.sync.dma_start(
                    out=qkbias,
                    in_=bass.AP(tensor=qkvb_d.ap().tensor, offset=0,
                                ap=[[0, 128], [1, 3 * C]]))

            hb = max(1, nblk // 2)          # blocks per half
            nh2 = hb * 128

            def emit_stats_p1(b, xt, c):
                    # ---------- GroupNorm stats, phase 1 (per c-tile) -------
                    # bn_stats + per-partition aggregation + the partition-
                    # gather DMA. Phase 2 is emitted later so the DMA
                    # round-trips of all four c-tiles overlap.
                    nsub = max(1, n // 512)
                    sd = nc.vector.BN_STATS_DIM
                    st = stats.tile([128, nsub, sd], F32, tag="bnst")
                    xv = xt[c].rearrange("p (s f) -> p s f", s=nsub)
                    for s in range(nsub):
                        nc.vector.bn_stats(out=st[:, s, :], in_=xv[:, s, :])
                    mv = stats.tile([128, nc.vector.BN_AGGR_DIM], F32, tag="bnmv")
                    nc.vector.bn_aggr(out=mv, in_=st)
                    # st2: col0 = mean, col1 = E[x^2] = var + mean^2
                    st2 = stats.tile([128, 2], F32, tag="st2")
                    nc.vector.tensor_copy(out=st2[:, 0:1], in_=mv[:, 0:1])
                    nc.vector.scalar_tensor_tensor(
                        out=st2[:, 1:2], in0=mv[:, 0:1], scalar=mv[:, 0:1],
                        in1=mv[:, 1:2], op0=ALU.mult, op1=ALU.add)
                    # gather all 128 partitions' stats onto one partition (DMA
                    # crosses partitions; avoids PE for the group aggregation)
                    stT = stats.tile([1, 256], F32, tag="stT")
                    nc.sync.dma_start(out=stT, in_=st2)
                    return stT

            def emit_stats_p2(b, stTs):
                    # ---------- GroupNorm stats, phase 2 ----------
                    scale_t, bias_t = [], []
                    for c in range(CT):
                        stT = stTs[c]
                        # per-group sums over the 16 channels: [1, 8, 2]
                        gsum = stats.tile([1, 16], F32, tag="gsum")
                        nc.vector.tensor_reduce(
                            out=gsum.rearrange("p (g c) -> p g c", g=8),
                            in_=_bc(stT, [(32, 8), (1, 2), (2, 16)]),
                            axis=AX.X, op=ALU.add)
                        gm = _bc(gsum, [(2, 8)])           # sum of means      [1,8]
                        ge = bass.AP(tensor=gsum.tensor, offset=gsum.offset + 1,
                                     ap=[list(gsum.ap[0])] + [[2, 8]])  # sum E[x^2]
                        m2 = stats.tile([1, 8], F32, tag="m2")
                        nc.vector.tensor_mul(m2, gm, gm)
                        # 256*var = 16*sum_ex2 - (sum_mean)^2
                        v256 = stats.tile([1, 8], F32, tag="v256")
                        nc.vector.scalar_tensor_tensor(
                            out=v256, in0=ge, scalar=16.0, in1=m2,
                            op0=ALU.mult, op1=ALU.subtract)
                        sg = stats.tile([1, 8], F32, tag="sg")
                        nc.scalar.activation(out=sg, in_=v256, func=ACTF.Sqrt,
                                             scale=1.0, bias=epst)   # sqrt(256(var+eps))
                        rg = stats.tile([1, 8], F32, tag="rg")
                        nc.vector.reciprocal(out=rg, in_=sg)          # rstd/16
                        # broadcast to 128 channel slots, interleaved (mean, rstd)
                        sb2 = stats.tile([1, 256], F32, tag="sb2")
                        nc.vector.tensor_scalar(
                            out=_bc(sb2, [(32, 8), (2, 16)]),
                            in0=_bc(gsum, [(2, 8), (0, 16)]), scalar1=1.0 / 16.0,
                            scalar2=None, op0=ALU.mult)
                        nc.vector.tensor_scalar(
                            out=bass.AP(tensor=sb2.tensor, offset=sb2.offset + 1,
                                        ap=[list(sb2.ap[0])] + [[32, 8], [2, 16]]),
                            in0=_bc(rg, [(1, 8), (0, 16)]), scalar1=16.0,
                            scalar2=None, op0=ALU.mult)
                        pb2 = stats.tile([128, 2], F32, tag="pb2")
                        nc.sync.dma_start(out=pb2, in_=sb2)
                        sc = scb.tile([128, 1], F32, tag="sc", name="sc")
                        bi = scb.tile([128, 1], F32, tag="bi", name="bi")
                        tmp = stats.tile([128, 1], F32, tag="tmp")
                        nc.vector.tensor_mul(sc, pb2[:, 1:2], nwt[c])
                        nc.vector.tensor_mul(tmp, pb2[:, 0:1], sc)
                        nc.vector.tensor_sub(bi, nbt[c], tmp)
                        scale_t.append(sc)
                        bias_t.append(bi)
                    return scale_t, bias_t

            def emit_norm(b, half, xt, scale_t, bias_t):
                hs = slice(half * nh2, min(n, (half + 1) * nh2))
                out = []
                for c in range(CT):
                    hp = hlo if half == 0 else hhi
                    t = hp.tile([128, nh2], BF16, tag=f"h{half}_{c}",
                                name=f"h{half}_{c}")
                    nc.scalar.activation(out=t, in_=xt[c][:, hs],
                                         func=ACTF.Identity,
                                         bias=bias_t[c], scale=scale_t[c])
                    out.append(t)
                    if debug and b == 0:
                        hf = stats.tile([128, nh2], F32, tag="dbgh", name="hf")
                        nc.vector.tensor_copy(out=hf, in_=t)
                        nc.sync.dma_start(
                            out=dbg["h"][c * 128:(c + 1) * 128, hs], in_=hf)
                return out

            def emit_stats(b, xt):
                return emit_stats_p2(b, [emit_stats_p1(b, xt, c)
                                         for c in range(CT)])

            st_cur = emit_stats(0, xt_cur)
            ht_cur = [emit_norm(0, 0, xt_cur, *st_cur), None]
            ht_cur[1] = emit_norm(0, 1, xt_cur, *st_cur) if nblk > 1 else ht_cur[0]
            for b in range(nb):
                xt, (scale_t, bias_t), ht = xt_cur, st_cur, ht_cur
                # ---------- per 128-position block ----------
                # The next batch's x load / stats / norm are emitted at fixed
                # points inside this loop so its head overlaps this batch's
                # attention tail (engine queues are in-order).
                nxt = {}
                bpc = cs // 128
                state = {"otcb": None}

                def emit_tail(tblk, qkv, s_l):
                    """Stage B of block tblk: softmax tail + AV + transpose +
                    proj. Emitted one block late so the ACT stream never
                    blocks the next block's qkv eviction behind exp()."""
                    v = qkv[:, 1024:1536]
                    # softmax over g: E = exp(S/8); logits bounded, no max-sub
                    e_l = spool.tile([128, NH * NH], BF16, tag="e")
                    nc.scalar.activation(out=e_l, in_=s_l, func=ACTF.Exp,
                                         scale=0.125)
                    d_l = spool.tile([128, NH], F32, tag="d")
                    nc.vector.tensor_reduce(
                        out=d_l, in_=e_l.rearrange("p (h g) -> p h g", g=NH),
                        axis=AX.X, op=ALU.add)
                    r_l = spool.tile([128, NH], F32, tag="r")
                    nc.vector.reciprocal(out=r_l, in_=d_l)
                    a_l = spool.tile([128, NH * NH], BF16, tag="a")
                    nc.vector.tensor_tensor(
                        out=a_l.rearrange("p (h g) -> p h g", g=NH),
                        in0=e_l.rearrange("p (h g) -> p h g", g=NH),
                        in1=_bc(r_l, [(1, NH), (0, NH)]),
                        op=ALU.mult)
                    # AV: U2[(h,d,g)] = A[h,g] * V'[d,g]; O = sum_g
                    # (V columns host-permuted to [d*8+g] so both reads are
                    # unit-stride innermost -> DVE 2x mode)
                    u2 = upool.tile([128, NH * HD * NH], BF16, tag="u")
                    nc.vector.tensor_tensor(
                        out=u2.rearrange("p (h d g) -> p h d g", h=NH, d=HD),
                        in0=_bc(a_l, [(NH, NH), (0, HD), (1, NH)]),
                        in1=_bc(v, [(0, NH), (NH, HD), (1, NH)]),
                        op=ALU.mult)
                    # O = sum_g via in-place halving adds down to TWO
                    # g-lanes (2x-mode eligible); the final pairwise add is
                    # folded into the PE transposes (PSUM accumulation), which
                    # saves a 1x count-1 DVE op per block.
                    uv = u2.rearrange("p (a g) -> p a g", g=NH)
                    w = NH
                    while w > 2:
                        nc.vector.tensor_tensor(
                            out=uv[:, :, 0:w // 2], in0=uv[:, :, 0:w // 2],
                            in1=uv[:, :, w // 2:w], op=ALU.add)
                        w //= 2
                    if debug and b == 0 and tblk == 0:
                        o_l = opool.tile([128, C], BF16, tag="o")
                        nc.vector.tensor_tensor(
                            out=o_l.rearrange("p (a u) -> p a u", u=1),
                            in0=uv[:, :, 0:1], in1=uv[:, :, 1:2], op=ALU.add)
                        for nm, src in (("qkv", qkv), ("s", s_l), ("a", a_l),
                                        ("o", o_l)):
                            ff = stats.tile(list(src.shape), F32,
                                            tag=f"dbg{nm}", name=f"f{nm}")
                            nc.vector.tensor_copy(out=ff, in_=src)
                            nc.sync.dma_start(out=dbg[nm][:, :], in_=ff)
                    return u2

                def emit_otproj(tblk, u2):
                    """Stage C of block tblk (two blocks late): transpose,
                    OT eviction, and the per-chunk proj+residual. Keeping this
                    off stage B decouples the next blocks' qkv evictions from
                    DVE's AV chain in the in-order ACT stream."""
                    if tblk % bpc == 0:
                        state["otcb"] = otr.tile([128, CT * cs], BF16,
                                                 tag="otr", name="otr")
                    otcb = state["otcb"]
                    # transpose O back to C-major, summing the two g-lanes
                    # via PSUM accumulation; one ACT eviction for all 4
                    # C-tiles (otcb columns [ob*cs + pos*128, +128))
                    pt = pmm.tile([128, 512], F32, tag="pt")
                    for ob in range(CT):
                        for gl in range(2):
                            lane = bass.AP(
                                tensor=u2.tensor,
                                offset=u2.offset + ob * 128 * NH + gl,
                                ap=[list(u2.ap[0]), [NH, 128]])
                            nc.tensor.matmul(
                                pt[:, ob * 128:(ob + 1) * 128], lane, ident,
                                start=(gl == 0), stop=(gl == 1))
                    pos = tblk % bpc
                    nc.scalar.copy(
                        out=bass.AP(tensor=otcb.tensor,
                                    offset=otcb.offset + pos * 128,
                                    ap=[list(otcb.ap[0]), [cs, CT], [1, 128]]),
                        in_=pt.rearrange("p (ob f) -> p ob f", ob=CT))

                    # proj + residual for chunk j once its OT columns exist.
                    # The residual is added in PSUM by an identity matmul over
                    # a bf16 copy of x (re-DMA'd per chunk so the x tiles die
                    # at norm time and the next batch's loads overlap);
                    # eviction adds proj_b on ACT.
                    if (tblk + 1) % bpc == 0:
                        j = tblk // bpc
                        ncs = slice(j * cs, (j + 1) * cs)
                        for c in range(CT):
                            xr = outsb.tile([128, cs], F32, tag="xr", name="xr")
                            nc.sync.dma_start(
                                out=xr, in_=x_d[b, c * 128:(c + 1) * 128, ncs])
                            xbf = outsb.tile([128, cs], BF16, tag="xbf",
                                             name="xbf")
                            nc.scalar.copy(out=xbf, in_=xr)
                            py = pmm2.tile([128, cs], F32, tag="py", name="py")
                            for ob in range(CT):
                                nc.tensor.matmul(
                                    py, pwT[ob][:, c * 128:(c + 1) * 128],
                                    otcb[:, ob * cs:(ob + 1) * cs],
                                    start=(ob == 0), stop=False)
                            nc.tensor.matmul(py, ident, xbf,
                                             start=False, stop=True)
                            ot = outsb.tile([128, cs], F32, tag="out",
                                            name="ot")
                            nc.scalar.activation(out=ot, in_=py,
                                                 func=ACTF.Identity,
                                                 bias=pbt[c], scale=1.0)
                            nc.sync.dma_start(
                                out=out_d[b, c * 128:(c + 1) * 128, ncs],
                                in_=ot)

                pend = None
                pend2 = None
                for blk in range(nblk):
                    if b + 1 < nb:
                        if blk == 4:
                            nxt["x"] = emit_xload(b + 1)
                        elif 16 <= blk < 16 + CT:
                            nxt.setdefault("stT", []).append(
                                emit_stats_p1(b + 1, nxt["x"], blk - 16))
                        elif blk == 21:
                            nxt["st"] = emit_stats_p2(b + 1, nxt["stT"])
                        elif blk == 24:
                            nxt["h0"] = emit_norm(b + 1, 0, nxt["x"], *nxt["st"])
                        elif blk == 28:
                            nxt["h1"] = emit_norm(b + 1, 1, nxt["x"], *nxt["st"])
                    half = min(blk // hb, 1)
                    hslice = slice(blk * 128 - half * nh2, (blk + 1) * 128 - half * nh2)
                    pq = pqkv.tile([128, 3 * C], F32, tag="pq", name="pq")
                    for c in range(CT):
                        lhsT = ht[half][c][:, hslice]
                        for oc in range(OT3):
                            nc.tensor.matmul(
                                pq[:, oc * 512:(oc + 1) * 512], lhsT,
                                wqkvT[c][:, oc * 512:(oc + 1) * 512],
                                start=(c == 0), stop=(c == CT - 1))
                    qkv = qkvsb.tile([128, 3 * C], BF16, tag="qkv")
                    if qkbias is not None:
                        nc.vector.tensor_add(out=qkv, in0=pq, in1=qkbias)
                    else:
                        nc.scalar.copy(out=qkv, in_=pq)

                    q = qkv[:, 0:512]
                    k = qkv[:, 512:1024]

                    # u1 multiply is split along d: the bulk runs on GPSIMD
                    # (TensorTensor — the only elementwise opcode GPSIMD
                    # codegen accepts), a small tail slice on DVE, sized so
                    # both engines pace at ~7.5us/block. The d-trees all stay
                    # on DVE where the 2x bf16 mode applies.
                    dp = HD - DSPLIT
                    # logits: U1[(h,g,d)] = q[h,d] * k[g,d]; S = sum over d
                    u1 = upool.tile([128, NH * NH * HD], BF16, tag="u")
                    s_l = spool.tile([128, NH * NH], F32, tag="s")
                    u1v = u1.rearrange("p (a d) -> p a d", d=HD)
                    u1hg = u1.rearrange("p (h g d) -> p h g d", h=NH, g=NH)
                    nc.gpsimd.tensor_tensor(
                        out=u1hg[:, :, :, 0:dp],
                        in0=_bc(q, [(HD, NH), (0, NH), (1, dp)]),
                        in1=_bc(k, [(0, NH), (HD, NH), (1, dp)]),
                        op=ALU.mult)
                    nc.vector.tensor_tensor(
                        out=u1hg[:, :, :, dp:HD],
                        in0=bass.AP(tensor=qkv.tensor,
                                    offset=qkv.offset + dp,
                                    ap=[list(qkv.ap[0]), [HD, NH], [0, NH],
                                        [1, DSPLIT]]),
                        in1=bass.AP(tensor=qkv.tensor,
                                    offset=qkv.offset + 512 + dp,
                                    ap=[list(qkv.ap[0]), [0, NH], [HD, NH],
                                        [1, DSPLIT]]),
                        op=ALU.mult)
                    w = HD
                    while w > 2:
                        nc.vector.tensor_tensor(
                            out=u1v[:, :, 0:w // 2], in0=u1v[:, :, 0:w // 2],
                            in1=u1v[:, :, w // 2:w], op=ALU.add)
                        w //= 2
                    nc.vector.tensor_tensor(
                        out=s_l.rearrange("p (a u) -> p a u", u=1),
                        in0=u1v[:, :, 0:1], in1=u1v[:, :, 1:2], op=ALU.add)
                    if pend is not None:
                        tb, tq, tsl = pend
                        ol = emit_tail(tb, tq, tsl)
                        if pend2 is not None:
                            emit_otproj(*pend2)
                        pend2 = (tb, ol)
                    pend = (blk, qkv, s_l)
                tb, tq, tsl = pend
                ol = emit_tail(tb, tq, tsl)
                if pend2 is not None:
                    emit_otproj(*pend2)
                emit_otproj(tb, ol)
                if b + 1 < nb:
                    if "x" not in nxt:
                        nxt["x"] = emit_xload(b + 1)
                    if "st" not in nxt:
                        nxt["st"] = emit_stats(b + 1, nxt["x"])
                    if "h0" not in nxt:
                        nxt["h0"] = emit_norm(b + 1, 0, nxt["x"], *nxt["st"])
                    if "h1" not in nxt:
                        nxt["h1"] = (emit_norm(b + 1, 1, nxt["x"], *nxt["st"])
                                     if nblk > 1 else nxt["h0"])
                    xt_cur, st_cur = nxt["x"], nxt["st"]
                    ht_cur = [nxt["h0"], nxt["h1"]]
    return nc


_CACHE = {}


def host_inputs(norm_w, norm_b, qkv_w, qkv_b, proj_w, proj_b):
    """Host-side weight preprocessing -> the kernel's shared input tensors."""
    bf = ml_dtypes.bfloat16
    # V-part column permutation: store V as [d*8+g] so the AV multiply reads
    # both operands at unit stride (DVE 2x mode).
    vperm = np.arange(3 * C)
    g_i, d_i = np.meshgrid(np.arange(NH), np.arange(HD), indexing="ij")
    vperm[2 * C:] = 2 * C + (d_i * NH + g_i).reshape(-1)   # old[g*64+d] -> new pos
    inv = np.empty_like(vperm)
    inv[vperm] = np.arange(3 * C)
    wq_p = qkv_w[inv]        # new column j holds old channel inv[j]
    qkvb_p = np.ascontiguousarray(qkv_b[inv])
    wqkvT = np.ascontiguousarray(wq_p.T).astype(bf)           # [C, 3C]
    pwT = np.ascontiguousarray(proj_w.T).astype(bf)           # [C(o), C(c)]
    ident = np.eye(128, dtype=np.float32).astype(bf)
    return dict(wqkvT=wqkvT, pwT=pwT,
                normw=np.asarray(norm_w, np.float32),
                normb=np.asarray(norm_b, np.float32),
                qkvb=qkvb_p, pbeff=np.asarray(proj_b, np.float32),
                ident=ident)


def kernel(x, norm_w, norm_b, qkv_w, qkv_b, proj_w, proj_b):
    x = np.asarray(x, np.float32)
    norm_w = np.asarray(norm_w, np.float32)
    norm_b = np.asarray(norm_b, np.float32)
    qkv_w = np.asarray(qkv_w, np.float32)
    qkv_b = np.asarray(qkv_b, np.float32)
    proj_w = np.asarray(proj_w, np.float32)
    proj_b = np.asarray(proj_b, np.float32)

    qk_bias = bool(np.any(qkv_b != 0))
    key = ("full", qk_bias)
    if key not in _CACHE:
        nc_new = build_kernel(qk_bias=qk_bias)
        _cap_sync_waits(nc_new)   # HW path only; CoreSim rejects bare NoOps
        _CACHE[key] = nc_new
    nc = _CACHE[key]

    shared = host_inputs(norm_w, norm_b, qkv_w, qkv_b, proj_w, proj_b)
    xs = x.reshape(B, C, N)
    in_maps = [dict(x=np.ascontiguousarray(xs[c * NB:(c + 1) * NB]), **shared)
               for c in range(NCORES)]
    res = run_bass_kernel_spmd(nc, in_maps, core_ids=list(range(NCORES)),
                               trace=bool(os.environ.get("KERNEL_TRACE")))
    global LAST_RES
    LAST_RES = res
    out = np.concatenate([res.results[c]["out"] for c in range(NCORES)], axis=0)
    return out.reshape(B, C, HH, WW).astype(np.float32)


LAST_RES = None



# revision 44
# speedup vs baseline: 1.4571x; 1.1473x over previous
"""Trainium2 Bass kernel for nn_AttentionBlock (GroupNorm + per-position
head-axis attention + proj + residual).

Sharding: data-parallel over batch B=16 -> 2 batches per core x 8 cores.
Each core runs an identical program on its x-shard [2, 512, 4096] plus
replicated (host-preprocessed) weights, and writes its out-shard.

Per-core pipeline:
  1. GroupNorm(32): bn_stats per partition over N; cross-partition group
     aggregation via two tiny SBUF->SBUF DMA gathers (DMA crosses
     partitions); normalize on ACT with per-partition scale/bias.
  2. QKV: out[n, o] via PE with h-block stationary -> QKV arrives N-major.
     h is normalized into two half-batch column groups (low half
     double-buffered) so consecutive batches overlap.
  3. Attention (N-major, per 128-position block): logits/AV as broadcast
     elementwise multiplies (bf16 unit-stride so the DVE 2x mode applies;
     the V weight columns are host-permuted to [d*8+g] for this). The
     logits multiply is split along d between GPSIMD (bulk) and DVE
     (DSPLIT tail columns) so both engines pace at ~5.5us/block; the
     d-reduction is an in-place halving add-tree on DVE (adds get 2x
     mode). Softmax skips max-subtraction (logits are O(1) by
     construction); Exp on ACT with the 1/8 scale folded in. The AV
     g-reduction costs no DVE time at all: each of the 8 g-lanes of U2 is
     transposed to C-major by an accumulating PE matmul (lhsT=lane,
     rhs=identity), summing in PSUM.
     The block loop is software-pipelined 3 deep (A: qkv+logits,
     B: softmax+AV one block later, C: transpose/proj one more block
     later) so the in-order ACT queue never blocks the next block's qkv
     eviction behind exp(), and Pool/DVE/ACT/PE all stream independently.
  4. proj matmul on PE consumes a 3-deep ring of per-chunk OT tiles; the
     residual is added in PSUM by an identity matmul over a bf16 copy of
     x (re-DMA'd per chunk); eviction adds proj_b on ACT; DMA out.
     The next batch's x load / GroupNorm stats / norm are emitted at
     fixed points inside the block loop so batches overlap.

Host-side preprocessing: weight transposes + bf16 casts + V-column permute.
If qkv_b is nonzero the kernel emits bias adds (specialized at trace; the
benchmark uses zero biases).

_cap_sync_waits: this walrus build accepts only ONE sync wait per compute
instruction; Tile emits more. The pass hoists excess waits onto same-engine
InstNoOps inserted immediately before the offender.
"""

import os

import numpy as np
import ml_dtypes

import concourse.bass as bass
import concourse.mybir as mybir
import concourse.tile as tile
from concourse.bass_utils import run_bass_kernel_spmd

F32 = mybir.dt.float32
BF16 = mybir.dt.bfloat16

B, C, HH, WW = 16, 512, 64, 64
N = HH * WW            # 4096
NB = 2                 # batches per core
NCORES = 8
NH, HD = 8, 64         # heads, head dim
GROUPS = 32
GSIZE = C // GROUPS    # 16 channels per group
EPS = 1e-5
CT = C // 128          # 4 channel tiles
OT3 = 3 * C // 512     # 3 o-chunks of 512 in qkv
NBLK = N // 128        # 32 position blocks per batch

AX = mybir.AxisListType
ALU = mybir.AluOpType
ACTF = mybir.ActivationFunctionType

# d-columns of each logits multiply computed on DVE (rest on GPSIMD)
DSPLIT = 8
QKV_BUFS = 4
U_BUFS = 4
S_BUFS = 4


def _bc(t, dims):
    """AP over tile/AP `t` with explicit free [step,count] dims (elem units)."""
    return bass.AP(tensor=t.tensor, offset=t.offset,
                   ap=[list(t.ap[0])] + [list(d) for d in dims])


def _cap_sync_waits(nc):
    """Walrus (this neuronxcc) allows at most 2 sync waits per compute
    instruction and is stricter still for some DMA structs. Tile can emit
    more. Hoist the excess onto a same-engine InstNoOp inserted immediately
    before the offender — the waits still complete before it executes."""
    import bass_rust
    n = 0
    for f in nc.m.functions:
        for blk in f.blocks:
            il = blk.instructions
            i = 0
            while i < len(il):
                ins = il[i]
                si = getattr(ins, "sync_info", None)
                if si is not None and si.on_wait and len(si.on_wait) > 1:
                    waits = list(si.on_wait)
                    for w in waits[:-1]:
                        nop = mybir.InstNoOp(name=f"W-abs-{n}", ins=[], outs=[])
                        n += 1
                        nop.engine = ins.engine
                        nop.sync_info = bass_rust.SyncInfo(on_wait=[w],
                                                           on_update=[])
                        il.insert(i, nop)
                        i += 1
                    si.on_wait = waits[-1:]
                i += 1
    return n


def build_kernel(nb=NB, nblk=NBLK, qk_bias=False, debug=False):
    n = nblk * 128
    cs = min(512, n)       # proj/residual n-chunk
    nch = n // cs
    nc = bass.Bass()
    dbg = {}
    if debug:
        dbg["h"] = nc.dram_tensor("dbg_h", [C, n], F32, kind="ExternalOutput")
        dbg["qkv"] = nc.dram_tensor("dbg_qkv", [128, 3 * C], F32, kind="ExternalOutput")
        dbg["s"] = nc.dram_tensor("dbg_s", [128, NH * NH], F32, kind="ExternalOutput")
        dbg["a"] = nc.dram_tensor("dbg_a", [128, NH * NH], F32, kind="ExternalOutput")
        dbg["o"] = nc.dram_tensor("dbg_o", [128, C], F32, kind="ExternalOutput")
        dbg["otsb"] = nc.dram_tensor("dbg_otsb", [C, n], F32, kind="ExternalOutput")

    x_d = nc.dram_tensor("x", [nb, C, n], F32, kind="ExternalInput")
    wqkvT_d = nc.dram_tensor("wqkvT", [C, 3 * C], BF16, kind="ExternalInput")
    pwT_d = nc.dram_tensor("pwT", [C, C], BF16, kind="ExternalInput")
    normw_d = nc.dram_tensor("normw", [C], F32, kind="ExternalInput")
    normb_d = nc.dram_tensor("normb", [C], F32, kind="ExternalInput")
    qkvb_d = nc.dram_tensor("qkvb", [3 * C], F32, kind="ExternalInput")
    pbeff_d = nc.dram_tensor("pbeff", [C], F32, kind="ExternalInput")
    ident_d = nc.dram_tensor("ident", [128, 128], BF16, kind="ExternalInput")
    out_d = nc.dram_tensor("out", [nb, C, n], F32, kind="ExternalOutput")

    with tile.TileContext(nc) as tc:
        with (
            tc.tile_pool(name="consts", bufs=1) as consts,
            tc.tile_pool(name="xpool", bufs=1) as xpool,
            tc.tile_pool(name="hlo", bufs=2) as hlo,
            tc.tile_pool(name="hhi", bufs=1) as hhi,
            tc.tile_pool(name="otr", bufs=3) as otr,
            tc.tile_pool(name="stats", bufs=2) as stats,
            tc.tile_pool(name="scb", bufs=4) as scb,
            tc.tile_pool(name="qkvsb", bufs=QKV_BUFS) as qkvsb,
            tc.tile_pool(name="upool", bufs=U_BUFS) as upool,
            tc.tile_pool(name="spool", bufs=S_BUFS) as spool,
            tc.tile_pool(name="opool", bufs=4) as opool,
            tc.tile_pool(name="outsb", bufs=2) as outsb,
            tc.tile_pool(name="pqkv", bufs=2, space="PSUM") as pqkv,   # 6 banks
            tc.tile_pool(name="pmm", bufs=1, space="PSUM") as pmm,     # 1 bank
            tc.tile_pool(name="pmm2", bufs=1, space="PSUM") as pmm2,   # 1 bank
        ):
            def emit_xload(b):
                xt = []
                for c in range(CT):
                    t = xpool.tile([128, n], F32, tag=f"x{c}")
                    nc.sync.dma_start(out=t, in_=x_d[b, c * 128:(c + 1) * 128, :])
                    xt.append(t)
                return xt

            # batch 0's x DMAs go first so GroupNorm stats start immediately;
            # the weight loads below overlap with them.
            xt_cur = emit_xload(0)

            # ---- constants / weights in SBUF ----
            wqkvT = []
            for c in range(CT):
                t = consts.tile([128, 3 * C], BF16, tag=f"wq{c}")
                nc.sync.dma_start(out=t, in_=wqkvT_d[c * 128:(c + 1) * 128, :])
                wqkvT.append(t)
            pwT = []
            for o in range(CT):
                t = consts.tile([128, C], BF16, tag=f"pw{o}")
                nc.sync.dma_start(out=t, in_=pwT_d[o * 128:(o + 1) * 128, :])
                pwT.append(t)
            ident = consts.tile([128, 128], BF16, tag="ident")
            nc.sync.dma_start(out=ident, in_=ident_d[:, :])
            nwt, nbt, pbt = [], [], []
            for c in range(CT):
                sl = slice(c * 128, (c + 1) * 128)
                t1 = consts.tile([128, 1], F32, tag=f"nw{c}")
                nc.sync.dma_start(out=t1, in_=normw_d[sl].rearrange("(p u) -> p u", u=1))
                nwt.append(t1)
                t2 = consts.tile([128, 1], F32, tag=f"nb{c}")
                nc.sync.dma_start(out=t2, in_=normb_d[sl].rearrange("(p u) -> p u", u=1))
                nbt.append(t2)
                t3 = consts.tile([128, 1], F32, tag=f"pb{c}")
                nc.sync.dma_start(out=t3, in_=pbeff_d[sl].rearrange("(p u) -> p u", u=1))
                pbt.append(t3)
            epst = consts.tile([1, 1], F32, tag="eps")
            nc.vector.memset(epst, 256.0 * EPS)
            qkbias = None
            if qk_bias:
                qkbias = consts.tile([128, 3 * C], F32, tag="qkb")
                nc.sync.dma_start(
                    out=qkbias,
                    in_=bass.AP(tensor=qkvb_d.ap().tensor, offset=0,
                                ap=[[0, 128], [1, 3 * C]]))

            hb = max(1, nblk // 2)          # blocks per half
            nh2 = hb * 128

            def emit_stats_p1(b, xt, c):
                    # ---------- GroupNorm stats, phase 1 (per c-tile) -------
                    # bn_stats + per-partition aggregation + the partition-
                    # gather DMA. Phase 2 is emitted later so the DMA
                    # round-trips of all four c-tiles overlap.
                    nsub = max(1, n // 512)
                    sd = nc.vector.BN_STATS_DIM
                    st = stats.tile([128, nsub, sd], F32, tag="bnst")
                    xv = xt[c].rearrange("p (s f) -> p s f", s=nsub)
                    for s in range(nsub):
                        nc.vector.bn_stats(out=st[:, s, :], in_=xv[:, s, :])
                    mv = stats.tile([128, nc.vector.BN_AGGR_DIM], F32, tag="bnmv")
                    nc.vector.bn_aggr(out=mv, in_=st)
                    # st2: col0 = mean, col1 = E[x^2] = var + mean^2
                    st2 = stats.tile([128, 2], F32, tag="st2")
                    nc.vector.tensor_copy(out=st2[:, 0:1], in_=mv[:, 0:1])
                    nc.vector.scalar_tensor_tensor(
                        out=st2[:, 1:2], in0=mv[:, 0:1], scalar=mv[:, 0:1],
                        in1=mv[:, 1:2], op0=ALU.mult, op1=ALU.add)
                    # gather all 128 partitions' stats onto one partition (DMA
                    # crosses partitions; avoids PE for the group aggregation)
                    stT = stats.tile([1, 256], F32, tag="stT")
                    nc.sync.dma_start(out=stT, in_=st2)
                    return stT

            def emit_stats_p2(b, stTs):
                    # ---------- GroupNorm stats, phase 2 ----------
                    scale_t, bias_t = [], []
                    for c in range(CT):
                        stT = stTs[c]
                        # per-group sums over the 16 channels: [1, 8, 2]
                        gsum = stats.tile([1, 16], F32, tag="gsum")
                        nc.vector.tensor_reduce(
                            out=gsum.rearrange("p (g c) -> p g c", g=8),
                            in_=_bc(stT, [(32, 8), (1, 2), (2, 16)]),
                            axis=AX.X, op=ALU.add)
                        gm = _bc(gsum, [(2, 8)])           # sum of means      [1,8]
                        ge = bass.AP(tensor=gsum.tensor, offset=gsum.offset + 1,
                                     ap=[list(gsum.ap[0])] + [[2, 8]])  # sum E[x^2]
                        m2 = stats.tile([1, 8], F32, tag="m2")
                        nc.vector.tensor_mul(m2, gm, gm)
                        # 256*var = 16*sum_ex2 - (sum_mean)^2
                        v256 = stats.tile([1, 8], F32, tag="v256")
                        nc.vector.scalar_tensor_tensor(
                            out=v256, in0=ge, scalar=16.0, in1=m2,
                            op0=ALU.mult, op1=ALU.subtract)
                        sg = stats.tile([1, 8], F32, tag="sg")
                        nc.scalar.activation(out=sg, in_=v256, func=ACTF.Sqrt,
                                             scale=1.0, bias=epst)   # sqrt(256(var+eps))
                        rg = stats.tile([1, 8], F32, tag="rg")
                        nc.vector.reciprocal(out=rg, in_=sg)          # rstd/16
                        # broadcast to 128 channel slots, interleaved (mean, rstd)
                        sb2 = stats.tile([1, 256], F32, tag="sb2")
                        nc.vector.tensor_scalar(
                            out=_bc(sb2, [(32, 8), (2, 16)]),
                            in0=_bc(gsum, [(2, 8), (0, 16)]), scalar1=1.0 / 16.0,
                            scalar2=None, op0=ALU.mult)
                        nc.vector.tensor_scalar(
                            out=bass.AP(tensor=sb2.tensor, offset=sb2.offset + 1,
                                        ap=[list(sb2.ap[0])] + [[32, 8], [2, 16]]),
                            in0=_bc(rg, [(1, 8), (0, 16)]), scalar1=16.0,
                            scalar2=None, op0=ALU.mult)
                        pb2 = stats.tile([128, 2], F32, tag="pb2")
                        nc.sync.dma_start(out=pb2, in_=sb2)
                        sc = scb.tile([128, 1], F32, tag="sc", name="sc")
                        bi = scb.tile([128, 1], F32, tag="bi", name="bi")
                        tmp = stats.tile([128, 1], F32, tag="tmp")
                        nc.vector.tensor_mul(sc, pb2[:, 1:2], nwt[c])
                        nc.vector.tensor_mul(tmp, pb2[:, 0:1], sc)
                        nc.vector.tensor_sub(bi, nbt[c], tmp)
                        scale_t.append(sc)
                        bias_t.append(bi)
                    return scale_t, bias_t

            def emit_norm(b, half, xt, scale_t, bias_t):
                hs = slice(half * nh2, min(n, (half + 1) * nh2))
                out = []
                for c in range(CT):
                    hp = hlo if half == 0 else hhi
                    t = hp.tile([128, nh2], BF16, tag=f"h{half}_{c}",
                                name=f"h{half}_{c}")
                    nc.scalar.activation(out=t, in_=xt[c][:, hs],
                                         func=ACTF.Identity,
                                         bias=bias_t[c], scale=scale_t[c])
                    out.append(t)
                    if debug and b == 0:
                        hf = stats.tile([128, nh2], F32, tag="dbgh", name="hf")
                        nc.vector.tensor_copy(out=hf, in_=t)
                        nc.sync.dma_start(
                            out=dbg["h"][c * 128:(c + 1) * 128, hs], in_=hf)
                return out

            def emit_stats(b, xt):
                return emit_stats_p2(b, [emit_stats_p1(b, xt, c)
                                         for c in range(CT)])

            st_cur = emit_stats(0, xt_cur)
            ht_cur = [emit_norm(0, 0, xt_cur, *st_cur), None]
            ht_cur[1] = emit_norm(0, 1, xt_cur, *st_cur) if nblk > 1 else ht_cur[0]
            for b in range(nb):
                xt, (scale_t, bias_t), ht = xt_cur, st_cur, ht_cur
                # ---------- per 128-position block ----------
                # The next batch's x load / stats / norm are emitted at fixed
                # points inside this loop so its head overlaps this batch's
                # attention tail (engine queues are in-order).
                nxt = {}
                bpc = cs // 128
                state = {"otcb": None}

                def emit_tail(tblk, qkv, s_l):
                    """Stage B of block tblk: softmax tail + AV + transpose +
                    proj. Emitted one block late so the ACT stream never
                    blocks the next block's qkv eviction behind exp()."""
                    v = qkv[:, 1024:1536]
                    # softmax over g: E = exp(S/8); logits bounded, no max-sub
                    e_l = spool.tile([128, NH * NH], BF16, tag="e")
                    nc.scalar.activation(out=e_l, in_=s_l, func=ACTF.Exp,
                                         scale=0.125)
                    d_l = spool.tile([128, NH], F32, tag="d")
                    nc.vector.tensor_reduce(
                        out=d_l, in_=e_l.rearrange("p (h g) -> p h g", g=NH),
                        axis=AX.X, op=ALU.add)
                    r_l = spool.tile([128, NH], F32, tag="r")
                    nc.vector.reciprocal(out=r_l, in_=d_l)
                    a_l = spool.tile([128, NH * NH], BF16, tag="a")
                    nc.vector.tensor_tensor(
                        out=a_l.rearrange("p (h g) -> p h g", g=NH),
                        in0=e_l.rearrange("p (h g) -> p h g", g=NH),
                        in1=_bc(r_l, [(1, NH), (0, NH)]),
                        op=ALU.mult)
                    # AV: U2[(h,d,g)] = A[h,g] * V'[d,g]; O = sum_g
                    # (V columns host-permuted to [d*8+g] so both reads are
                    # unit-stride innermost -> DVE 2x mode)
                    u2 = upool.tile([128, NH * HD * NH], BF16, tag="u")
                    nc.vector.tensor_tensor(
                        out=u2.rearrange("p (h d g) -> p h d g", h=NH, d=HD),
                        in0=_bc(a_l, [(NH, NH), (0, HD), (1, NH)]),
                        in1=_bc(v, [(0, NH), (NH, HD), (1, NH)]),
                        op=ALU.mult)
                    # O = sum_g is folded ENTIRELY into the PE transposes:
                    # each of the 8 g-lanes is transposed with PSUM
                    # accumulation, so no DVE reduction tree is needed.
                    uv = u2.rearrange("p (a g) -> p a g", g=NH)
                    if debug and b == 0 and tblk == 0:
                        o_l = opool.tile([128, C], BF16, tag="o")
                        nc.vector.tensor_reduce(
                            out=o_l, in_=uv, axis=AX.X, op=ALU.add)
                        for nm, src in (("qkv", qkv), ("s", s_l), ("a", a_l),
                                        ("o", o_l)):
                            ff = stats.tile(list(src.shape), F32,
                                            tag=f"dbg{nm}", name=f"f{nm}")
                            nc.vector.tensor_copy(out=ff, in_=src)
                            nc.sync.dma_start(out=dbg[nm][:, :], in_=ff)
                    return u2

                def emit_otproj(tblk, u2):
                    """Stage C of block tblk (two blocks late): transpose,
                    OT eviction, and the per-chunk proj+residual. Keeping this
                    off stage B decouples the next blocks' qkv evictions from
                    DVE's AV chain in the in-order ACT stream."""
                    if tblk % bpc == 0:
                        state["otcb"] = otr.tile([128, CT * cs], BF16,
                                                 tag="otr", name="otr")
                    otcb = state["otcb"]
                    # transpose O back to C-major, summing the two g-lanes
                    # via PSUM accumulation; one ACT eviction for all 4
                    # C-tiles (otcb columns [ob*cs + pos*128, +128))
                    pt = pmm.tile([128, 512], F32, tag="pt")
                    for ob in range(CT):
                        for gl in range(NH):
                            lane = bass.AP(
                                tensor=u2.tensor,
                                offset=u2.offset + ob * 128 * NH + gl,
                                ap=[list(u2.ap[0]), [NH, 128]])
                            nc.tensor.matmul(
                                pt[:, ob * 128:(ob + 1) * 128], lane, ident,
                                start=(gl == 0), stop=(gl == NH - 1))
                    pos = tblk % bpc
                    nc.scalar.copy(
                        out=bass.AP(tensor=otcb.tensor,
                                    offset=otcb.offset + pos * 128,
                                    ap=[list(otcb.ap[0]), [cs, CT], [1, 128]]),
                        in_=pt.rearrange("p (ob f) -> p ob f", ob=CT))

                    # proj + residual for chunk j once its OT columns exist.
                    # The residual is added in PSUM by an identity matmul over
                    # a bf16 copy of x (re-DMA'd per chunk so the x tiles die
                    # at norm time and the next batch's loads overlap);
                    # eviction adds proj_b on ACT.
                    if (tblk + 1) % bpc == 0:
                        j = tblk // bpc
                        ncs = slice(j * cs, (j + 1) * cs)
                        for c in range(CT):
                            xr = outsb.tile([128, cs], F32, tag="xr", name="xr")
                            nc.sync.dma_start(
                                out=xr, in_=x_d[b, c * 128:(c + 1) * 128, ncs])
                            xbf = outsb.tile([128, cs], BF16, tag="xbf",
                                             name="xbf")
                            nc.scalar.copy(out=xbf, in_=xr)
                            py = pmm2.tile([128, cs], F32, tag="py", name="py")
                            for ob in range(CT):
                                nc.tensor.matmul(
                                    py, pwT[ob][:, c * 128:(c + 1) * 128],
                                    otcb[:, ob * cs:(ob + 1) * cs],
                                    start=(ob == 0), stop=False)
                            nc.tensor.matmul(py, ident, xbf,
                                             start=False, stop=True)
                            ot = outsb.tile([128, cs], F32, tag="out",
                                            name="ot")
                            nc.scalar.activation(out=ot, in_=py,
                                                 func=ACTF.Identity,
                                                 bias=pbt[c], scale=1.0)
                            nc.sync.dma_start(
                                out=out_d[b, c * 128:(c + 1) * 128, ncs],
                                in_=ot)

                pend = None
                pend2 = None
                for blk in range(nblk):
                    if b + 1 < nb:
                        if blk == 4:
                            nxt["x"] = emit_xload(b + 1)
                        elif 16 <= blk < 16 + CT:
                            nxt.setdefault("stT", []).append(
                                emit_stats_p1(b + 1, nxt["x"], blk - 16))
                        elif blk == 21:
                            nxt["st"] = emit_stats_p2(b + 1, nxt["stT"])
                        elif blk == 24:
                            nxt["h0"] = emit_norm(b + 1, 0, nxt["x"], *nxt["st"])
                        elif blk == 28:
                            nxt["h1"] = emit_norm(b + 1, 1, nxt["x"], *nxt["st"])
                    half = min(blk // hb, 1)
                    hslice = slice(blk * 128 - half * nh2, (blk + 1) * 128 - half * nh2)
                    pq = pqkv.tile([128, 3 * C], F32, tag="pq", name="pq")
                    for c in range(CT):
                        lhsT = ht[half][c][:, hslice]
                        for oc in range(OT3):
                            nc.tensor.matmul(
                                pq[:, oc * 512:(oc + 1) * 512], lhsT,
                                wqkvT[c][:, oc * 512:(oc + 1) * 512],
                                start=(c == 0), stop=(c == CT - 1))
                    qkv = qkvsb.tile([128, 3 * C], BF16, tag="qkv")
                    if qkbias is not None:
                        nc.vector.tensor_add(out=qkv, in0=pq, in1=qkbias)
                    else:
                        nc.scalar.copy(out=qkv, in_=pq)

                    q = qkv[:, 0:512]
                    k = qkv[:, 512:1024]

                    # u1 multiply is split along d: the bulk runs on GPSIMD
                    # (TensorTensor — the only elementwise opcode GPSIMD
                    # codegen accepts), a small tail slice on DVE, sized so
                    # both engines pace at ~7.5us/block. The d-trees all stay
                    # on DVE where the 2x bf16 mode applies.
                    dp = HD - DSPLIT
                    # logits: U1[(h,g,d)] = q[h,d] * k[g,d]; S = sum over d
                    u1 = upool.tile([128, NH * NH * HD], BF16, tag="u")
                    s_l = spool.tile([128, NH * NH], F32, tag="s")
                    u1v = u1.rearrange("p (a d) -> p a d", d=HD)
                    u1hg = u1.rearrange("p (h g d) -> p h g d", h=NH, g=NH)
                    nc.gpsimd.tensor_tensor(
                        out=u1hg[:, :, :, 0:dp],
                        in0=_bc(q, [(HD, NH), (0, NH), (1, dp)]),
                        in1=_bc(k, [(0, NH), (HD, NH), (1, dp)]),
                        op=ALU.mult)
                    nc.vector.tensor_tensor(
                        out=u1hg[:, :, :, dp:HD],
                        in0=bass.AP(tensor=qkv.tensor,
                                    offset=qkv.offset + dp,
                                    ap=[list(qkv.ap[0]), [HD, NH], [0, NH],
                                        [1, DSPLIT]]),
                        in1=bass.AP(tensor=qkv.tensor,
                                    offset=qkv.offset + 512 + dp,
                                    ap=[list(qkv.ap[0]), [0, NH], [HD, NH],
                                        [1, DSPLIT]]),
                        op=ALU.mult)
                    w = HD
                    while w > 2:
                        nc.vector.tensor_tensor(
                            out=u1v[:, :, 0:w // 2], in0=u1v[:, :, 0:w // 2],
                            in1=u1v[:, :, w // 2:w], op=ALU.add)
                        w //= 2
                    nc.vector.tensor_tensor(
                        out=s_l.rearrange("p (a u) -> p a u", u=1),
                        in0=u1v[:, :, 0:1], in1=u1v[:, :, 1:2], op=ALU.add)
                    if pend is not None:
                        tb, tq, tsl = pend
                        ol = emit_tail(tb, tq, tsl)
                        if pend2 is not None:
                            emit_otproj(*pend2)
                        pend2 = (tb, ol)
                    pend = (blk, qkv, s_l)
                tb, tq, tsl = pend
                ol = emit_tail(tb, tq, tsl)
                if pend2 is not None:
                    emit_otproj(*pend2)
                emit_otproj(tb, ol)
                if b + 1 < nb:
                    if "x" not in nxt:
                        nxt["x"] = emit_xload(b + 1)
                    if "st" not in nxt:
                        nxt["st"] = emit_stats(b + 1, nxt["x"])
                    if "h0" not in nxt:
                        nxt["h0"] = emit_norm(b + 1, 0, nxt["x"], *nxt["st"])
                    if "h1" not in nxt:
                        nxt["h1"] = (emit_norm(b + 1, 1, nxt["x"], *nxt["st"])
                                     if nblk > 1 else nxt["h0"])
                    xt_cur, st_cur = nxt["x"], nxt["st"]
                    ht_cur = [nxt["h0"], nxt["h1"]]
    return nc


_CACHE = {}


def host_inputs(norm_w, norm_b, qkv_w, qkv_b, proj_w, proj_b):
    """Host-side weight preprocessing -> the kernel's shared input tensors."""
    bf = ml_dtypes.bfloat16
    # V-part column permutation: store V as [d*8+g] so the AV multiply reads
    # both operands at unit stride (DVE 2x mode).
    vperm = np.arange(3 * C)
    g_i, d_i = np.meshgrid(np.arange(NH), np.arange(HD), indexing="ij")
    vperm[2 * C:] = 2 * C + (d_i * NH + g_i).reshape(-1)   # old[g*64+d] -> new pos
    inv = np.empty_like(vperm)
    inv[vperm] = np.arange(3 * C)
    wq_p = qkv_w[inv]        # new column j holds old channel inv[j]
    qkvb_p = np.ascontiguousarray(qkv_b[inv])
    wqkvT = np.ascontiguousarray(wq_p.T).astype(bf)           # [C, 3C]
    pwT = np.ascontiguousarray(proj_w.T).astype(bf)           # [C(o), C(c)]
    ident = np.eye(128, dtype=np.float32).astype(bf)
    return dict(wqkvT=wqkvT, pwT=pwT,
                normw=np.asarray(norm_w, np.float32),
                normb=np.asarray(norm_b, np.float32),
                qkvb=qkvb_p, pbeff=np.asarray(proj_b, np.float32),
                ident=ident)


def kernel(x, norm_w, norm_b, qkv_w, qkv_b, proj_w, proj_b):
    x = np.asarray(x, np.float32)
    norm_w = np.asarray(norm_w, np.float32)
    norm_b = np.asarray(norm_b, np.float32)
    qkv_w = np.asarray(qkv_w, np.float32)
    qkv_b = np.asarray(qkv_b, np.float32)
    proj_w = np.asarray(proj_w, np.float32)
    proj_b = np.asarray(proj_b, np.float32)

    qk_bias = bool(np.any(qkv_b != 0))
    key = ("full", qk_bias)
    if key not in _CACHE:
        nc_new = build_kernel(qk_bias=qk_bias)
        _cap_sync_waits(nc_new)   # HW path only; CoreSim rejects bare NoOps
        _CACHE[key] = nc_new
    nc = _CACHE[key]

    shared = host_inputs(norm_w, norm_b, qkv_w, qkv_b, proj_w, proj_b)
    xs = x.reshape(B, C, N)
    in_maps = [dict(x=np.ascontiguousarray(xs[c * NB:(c + 1) * NB]), **shared)
               for c in range(NCORES)]
    res = run_bass_kernel_spmd(nc, in_maps, core_ids=list(range(NCORES)),
                               trace=bool(os.environ.get("KERNEL_TRACE")))
    global LAST_RES
    LAST_RES = res
    out = np.concatenate([res.results[c]["out"] for c in range(NCORES)], axis=0)
    return out.reshape(B, C, HH, WW).astype(np.float32)


LAST_RES = None

